# revision 27
# baseline (speedup 1.0000x reference)
"""TRN2 Bass kernel for nn_GCNBasic (2-layer GCN, B=32, N=2048, F=128, H=256).

Sharding: data-parallel over batch B across 8 NeuronCores (4 items/core);
small weights replicated.  A_hat is transposed and cast to bf16 on the HOST
(layout prep, halves HBM traffic); the device streams A^T tiles straight
into SBUF and runs pure matmul pipelines:

  (AX)^T[f,n]  = sum_mb  X[mb]-stationary   @ A^T[mb]   (c-outer, 4 psum
                                                         512-chunks live)
  H1pre[n,h]   = (AX)^T[:,nb]-stationary    @ W1
  H1           = relu(LN(H1pre + b1))        (bias-add Pool, sum DVE,
                                              sumsq+apply ACT, f32 stats)
  (AH)^T[hh,n] = sum_mb H1[mb,hh]-stationary @ A^T[mb]   (hh outer)
  H2pre[n,k]   = sum_hh (AH)^T[hh,nb]-stat.  @ diag(g1)W2
  H2           = relu(LN(H2pre + b2))
  g^T          = sum_nb H2[nb,kh]-stationary @ ones  (mean pool via PE)
  outputs      = diag(g2)Wa/Wl heads in fp32, biases added on ACT.

Items are software-pipelined (load it+2 / L1 of it+1 emitted between L2
and pool of it) so PE never waits on LayerNorm barriers or DMA refill.

gamma folds (diag(g1)@W2, diag(g2)@Wa/Wl) are exact because relu(g*z)=
g*relu(z) for g>0; beta==0 fast path (the problem's setup_inputs always
produces gamma=1, beta=0); a general gamma/beta path exists as a fallback.

Known TRN2 pitfalls worked around here: tensor_tensor_reduce crashes the
device; ACT/DVE writes into PSUM are unstable -> squares go to SBUF scratch.
"""

from contextlib import ExitStack

import numpy as np
import ml_dtypes

import concourse.bacc as bacc
import concourse.mybir as mybir
import concourse.tile as tile
from concourse.bass_utils import run_bass_kernel_spmd

F32 = mybir.dt.float32
BF16 = mybir.dt.bfloat16
bf16 = ml_dtypes.bfloat16

N = 2048
F = 128
H = 256
K = 64
P = 128
NB = N // P
NCH = N // 512
EPS = 1e-5
N_CORES = 8


def _declare_io(nc, items, general):
    io = {}
    io["at4"] = nc.dram_tensor("at4", [items, N, N], BF16, kind="ExternalInput")
    io["x4"] = nc.dram_tensor("x4", [items, N, F], BF16, kind="ExternalInput")
    # W1/W2 carry an extra trailing column holding W@1 so the dense matmul
    # also produces the LayerNorm row-sum (sans bias) as output column H.
    io["w1"] = nc.dram_tensor("w1", [F, H + 1], BF16, kind="ExternalInput")
    io["w2"] = nc.dram_tensor("w2", [H, H + 1], BF16, kind="ExternalInput")
    io["b1bc"] = nc.dram_tensor("b1bc", [P, H], F32, kind="ExternalInput")
    io["b2bc"] = nc.dram_tensor("b2bc", [P, H], F32, kind="ExternalInput")
    io["sb1"] = nc.dram_tensor("sb1", [P, 1], F32, kind="ExternalInput")
    io["sb2"] = nc.dram_tensor("sb2", [P, 1], F32, kind="ExternalInput")
    io["wa"] = nc.dram_tensor("wa", [H, K], F32, kind="ExternalInput")
    io["wl"] = nc.dram_tensor("wl", [H, K], F32, kind="ExternalInput")
    io["ba"] = nc.dram_tensor("ba", [K, 1], F32, kind="ExternalInput")
    io["bl"] = nc.dram_tensor("bl", [K, 1], F32, kind="ExternalInput")
    io["ones"] = nc.dram_tensor("ones", [P, 1], BF16, kind="ExternalInput")
    if general:
        io["g1bc"] = nc.dram_tensor("g1bc", [P, H], F32, kind="ExternalInput")
        io["be1bc"] = nc.dram_tensor("be1bc", [P, H], F32, kind="ExternalInput")
        io["g2bc"] = nc.dram_tensor("g2bc", [P, H], F32, kind="ExternalInput")
        io["be2bc"] = nc.dram_tensor("be2bc", [P, H], F32, kind="ExternalInput")
    io["op"] = nc.dram_tensor("op", [items, K], F32, kind="ExternalOutput")
    io["ol"] = nc.dram_tensor("ol", [items, K], F32, kind="ExternalOutput")
    return io


def _build_core(nc, tc, io, items, general):
    at4, x4 = io["at4"], io["x4"]
    es = ExitStack()

    consts = es.enter_context(tc.tile_pool(name="consts", bufs=1))
    wts = es.enter_context(tc.tile_pool(name="wts", bufs=1))
    pool_at = es.enter_context(tc.tile_pool(name="at", bufs=2 * NB))
    pool_xb = es.enter_context(tc.tile_pool(name="xb", bufs=2))
    pool_axT = es.enter_context(tc.tile_pool(name="axT", bufs=2))
    pool_h1 = es.enter_context(tc.tile_pool(name="h1", bufs=2))
    pool_ahT = es.enter_context(tc.tile_pool(name="ahT", bufs=2))
    pool_h2 = es.enter_context(tc.tile_pool(name="h2", bufs=1))
    pool_hc = es.enter_context(tc.tile_pool(name="hc", bufs=NB))
    pool_sq = es.enter_context(tc.tile_pool(name="sq", bufs=2))
    pool_st = es.enter_context(tc.tile_pool(name="st", bufs=4))
    pool_gsb = es.enter_context(tc.tile_pool(name="gsb", bufs=4))
    pool_osb = es.enter_context(tc.tile_pool(name="osb", bufs=4))

    ps_big = es.enter_context(tc.tile_pool(name="ps_big", bufs=6, space="PSUM"))
    ps_h = es.enter_context(tc.tile_pool(name="ps_h", bufs=2, space="PSUM"))
    ps_sm = ps_h  # pg/po share the ps_h banks (never live at the same time)

    eps_t = consts.tile([P, 1], F32)
    nc.vector.memset(eps_t[:], EPS)
    ones_b = consts.tile([P, 1], BF16)
    w1_t = wts.tile([P, H + 1], BF16)
    w2_t = [wts.tile([P, H + 1], BF16, tag=f"w2_{hh}", name=f"w2_{hh}")
            for hh in range(2)]
    b1_t = wts.tile([P, H], F32)
    b2_t = wts.tile([P, H], F32)
    sb1_t = wts.tile([P, 1], F32)
    sb2_t = wts.tile([P, 1], F32)
    wa_t = [wts.tile([P, K], F32, tag=f"wa_{hh}", name=f"wa_{hh}")
            for hh in range(2)]
    wl_t = [wts.tile([P, K], F32, tag=f"wl_{hh}", name=f"wl_{hh}")
            for hh in range(2)]
    ba_t = wts.tile([K, 1], F32)
    bl_t = wts.tile([K, 1], F32)
    gb_t = {}
    if general:
        for nm in ("g1bc", "be1bc", "g2bc", "be2bc"):
            gb_t[nm] = wts.tile([P, H], F32, tag=nm, name=nm)

    def emit_weight_dmas():
        nc.sync.dma_start(ones_b[:], io["ones"][:])
        nc.sync.dma_start(w1_t[:], io["w1"][:])
        for hh in range(2):
            nc.sync.dma_start(w2_t[hh][:], io["w2"][hh * P:(hh + 1) * P, :])
        nc.sync.dma_start(b1_t[:], io["b1bc"][:])
        nc.sync.dma_start(b2_t[:], io["b2bc"][:])
        nc.sync.dma_start(sb1_t[:], io["sb1"][:])
        nc.sync.dma_start(sb2_t[:], io["sb2"][:])
        for hh in range(2):
            nc.sync.dma_start(wa_t[hh][:], io["wa"][hh * P:(hh + 1) * P, :])
            nc.sync.dma_start(wl_t[hh][:], io["wl"][hh * P:(hh + 1) * P, :])
        nc.sync.dma_start(ba_t[:], io["ba"][:])
        nc.sync.dma_start(bl_t[:], io["bl"][:])
        for nm, t in gb_t.items():
            nc.sync.dma_start(t[:], io[nm][:])

    inv_h = 1.0 / H

    # per-item live tiles (indexed by item)
    at_t = [None] * items
    xb_t = [None] * items
    axT_t = [None] * items
    h1_t = [None] * items
    ahT_t = [None] * items
    h2_t = [None] * items
    hc1_t = [None] * items
    hc2_t = [None] * items
    st1_t = [None] * items
    st2_t = [None] * items

    def load(it, chunks=1):
        xb = pool_xb.tile([P, NB, F], BF16, tag="xb", name=f"xb_{it}")
        nc.sync.dma_start(xb[:], x4[it].rearrange("(c p) f -> p c f", p=P))
        xb_t[it] = xb
        ats = [pool_at.tile([P, N], BF16, tag="at", name=f"at_{it}_{c}")
               for c in range(NB)]
        cw = N // chunks
        for c in range(NB):
            for k in range(chunks):
                nc.sync.dma_start(
                    ats[c][:, k * cw:(k + 1) * cw],
                    at4[it, c * P:(c + 1) * P, k * cw:(k + 1) * cw])
        at_t[it] = ats

    def l1_agg(it, unit=None):
        # one pending dense unit (PE matmul + LN stats) dripped per c
        # iteration so the LN chain paces alongside pure agg matmuls
        at, xb = at_t[it], xb_t[it]
        pb = [ps_big.tile([P, 512], F32, tag="big", name=f"ax_{it}_{j}")
              for j in range(NCH)]
        for c in range(NB):
            for j in range(NCH):
                nc.tensor.matmul(pb[j][:], xb[:, c, :],
                                 at[c][:, j * 512:(j + 1) * 512],
                                 start=(c == 0), stop=(c == NB - 1))
            if unit is not None:
                unit(c)
        axT = pool_axT.tile([P, N], BF16, tag="axT", name=f"axT_{it}")
        for j in range(NCH):
            if j % 2 == 0:
                nc.scalar.copy(axT[:, j * 512:(j + 1) * 512], pb[j][:])
            else:
                nc.vector.tensor_copy(axT[:, j * 512:(j + 1) * 512], pb[j][:])
        axT_t[it] = axT

    def ln_stats(nb, ph, b_t, sb_t, st, hc, sfx):
        # bias add on DVE (PSUM->SBUF); row-sum comes from the matmul's
        # extra column H (+ bias total from sb); sumsq on Pool (one
        # scalar_tensor_tensor with accum_out -- keeps ACT free for applies)
        nc.vector.tensor_tensor(out=hc[:], in0=ph[:, 0:H], in1=b_t[:],
                                op=mybir.AluOpType.add)
        nc.vector.tensor_tensor(out=st[:, 0, nb:nb + 1], in0=ph[:, H:H + 1],
                                in1=sb_t[:], op=mybir.AluOpType.add)
        sq = pool_sq.tile([P, H], F32, tag="sq", name=f"sq_{sfx}")
        nc.scalar.activation(
            out=sq[:], in_=hc[:], func=mybir.ActivationFunctionType.Square,
            accum_out=st[:, 1, nb:nb + 1])

    def finish_stats(st):
        s = st
        nc.gpsimd.tensor_scalar(out=s[:, 2, :], in0=s[:, 0, :],
                                scalar1=-inv_h, scalar2=None,
                                op0=mybir.AluOpType.mult)          # -mu
        nc.gpsimd.tensor_tensor(out=s[:, 3, :], in0=s[:, 2, :], in1=s[:, 2, :],
                                op=mybir.AluOpType.mult)           # mu^2
        nc.gpsimd.tensor_scalar(out=s[:, 4, :], in0=s[:, 1, :],
                                scalar1=inv_h, scalar2=None,
                                op0=mybir.AluOpType.mult)          # E[x^2]
        nc.gpsimd.tensor_tensor(out=s[:, 4, :], in0=s[:, 4, :], in1=s[:, 3, :],
                                op=mybir.AluOpType.subtract)       # var
        nc.scalar.activation(out=s[:, 5, :], in_=s[:, 4, :],
                             func=mybir.ActivationFunctionType.Sqrt,
                             bias=eps_t[:], scale=1.0)             # sd
        nc.vector.reciprocal(out=s[:, 6, :], in_=s[:, 5, :])       # 1/sd
        nc.gpsimd.tensor_tensor(out=s[:, 7, :], in0=s[:, 2, :], in1=s[:, 6, :],
                                op=mybir.AluOpType.mult)           # -mu/sd

    def apply_ln(nb, hc, st, h_out, g_bc, be_bc, use_act=False):
        if not general:
            if use_act:
                nc.scalar.activation(out=h_out, in_=hc[:],
                                     func=mybir.ActivationFunctionType.Relu,
                                     bias=st[:, 7, nb:nb + 1],
                                     scale=st[:, 6, nb:nb + 1])
            else:
                # Pool two-op apply keeps ACT free for the squares
                nc.gpsimd.tensor_scalar(out=hc[:], in0=hc[:],
                                        scalar1=st[:, 6, nb:nb + 1],
                                        scalar2=st[:, 7, nb:nb + 1],
                                        op0=mybir.AluOpType.mult,
                                        op1=mybir.AluOpType.add)
                nc.gpsimd.tensor_scalar_max(h_out, hc[:], 0.0)
        else:
            nc.scalar.activation(out=hc[:], in_=hc[:],
                                 func=mybir.ActivationFunctionType.Identity,
                                 bias=st[:, 7, nb:nb + 1],
                                 scale=st[:, 6, nb:nb + 1])
            nc.gpsimd.tensor_tensor(out=hc[:], in0=hc[:], in1=g_bc[:],
                                    op=mybir.AluOpType.mult)
            nc.vector.tensor_tensor(out=hc[:], in0=hc[:], in1=be_bc[:],
                                    op=mybir.AluOpType.add)
            nc.scalar.activation(out=h_out, in_=hc[:],
                                 func=mybir.ActivationFunctionType.Relu)

    def l2_agg(it, unit=None):
        at, h1 = at_t[it], h1_t[it]
        ahT = [pool_ahT.tile([P, N], BF16, tag="ahT", name=f"ahT_{it}_{hh}")
               for hh in range(2)]
        for hh in range(2):
            pb = [ps_big.tile([P, 512], F32, tag="big",
                              name=f"ah_{it}_{hh}_{j}") for j in range(NCH)]
            for c in range(NB):
                for j in range(NCH):
                    nc.tensor.matmul(pb[j][:], h1[:, c, hh * P:(hh + 1) * P],
                                     at[c][:, j * 512:(j + 1) * 512],
                                     start=(c == 0), stop=(c == NB - 1))
                if unit is not None and hh == 0:
                    unit(c)
            for j in range(NCH):
                if j % 2 == 0:
                    nc.scalar.copy(ahT[hh][:, j * 512:(j + 1) * 512], pb[j][:])
                else:
                    nc.vector.tensor_copy(
                        ahT[hh][:, j * 512:(j + 1) * 512], pb[j][:])
        ahT_t[it] = ahT

    pg_t = [None] * items

    def make_l1_dense(it):
        axT = axT_t[it]
        st1 = pool_st.tile([P, 8, NB], F32, tag="st", name=f"st1_{it}")
        h1 = pool_h1.tile([P, NB, H], BF16, tag="h1", name=f"h1_{it}")
        hc1 = []
        st1_t[it], h1_t[it] = st1, h1

        def unit(nb):
            ph = ps_h.tile([P, H + 1], F32, tag="h", name=f"p1_{it}_{nb}")
            nc.tensor.matmul(ph[:], axT[:, nb * P:(nb + 1) * P], w1_t[:],
                             start=True, stop=True)
            hc = pool_hc.tile([P, H], F32, tag="hc", name=f"hc1_{it}_{nb}")
            ln_stats(nb, ph, b1_t, sb1_t, st1, hc, f"1_{it}_{nb}")
            hc1.append(hc)

        def fin():
            finish_stats(st1)
            for nb in range(NB):
                apply_ln(nb, hc1[nb], st1, h1[:, nb, :],
                         gb_t.get("g1bc"), gb_t.get("be1bc"))

        return unit, fin

    def make_l2_dense(it, fuse_pool=False):
        ahT = ahT_t[it]
        st2 = pool_st.tile([P, 8, NB], F32, tag="st", name=f"st2_{it}")
        h2 = pool_h2.tile([P, NB, H], BF16, tag="h2", name=f"h2_{it}")
        hc2 = []
        st2_t[it], h2_t[it] = st2, h2

        def unit(nb):
            ph = ps_h.tile([P, H + 1], F32, tag="h", name=f"p2_{it}_{nb}")
            for hh in range(2):
                nc.tensor.matmul(ph[:], ahT[hh][:, nb * P:(nb + 1) * P],
                                 w2_t[hh][:], start=(hh == 0), stop=(hh == 1))
            hc = pool_hc.tile([P, H], F32, tag="hc", name=f"hc2_{it}_{nb}")
            ln_stats(nb, ph, b2_t, sb2_t, st2, hc, f"2_{it}_{nb}")
            hc2.append(hc)

        def fin():
            finish_stats(st2)
            if fuse_pool:
                # last item: interleave mean-pool accumulation with the
                # applies so the PE doesn't sit out the whole LN2 tail;
                # applies alternate ACT / Pool to halve the serial chain
                pg = [ps_sm.tile([P, 1], F32, tag="h", name=f"pg_{it}_{kh}")
                      for kh in range(2)]
                for nb in range(NB):
                    apply_ln(nb, hc2[nb], st2, h2[:, nb, :],
                             gb_t.get("g2bc"), gb_t.get("be2bc"),
                             use_act=(nb % 2 == 0))
                    for kh in range(2):
                        nc.tensor.matmul(pg[kh][:],
                                         h2[:, nb, kh * P:(kh + 1) * P],
                                         ones_b[:], start=(nb == 0),
                                         stop=(nb == NB - 1))
                pg_t[it] = pg
            else:
                for nb in range(NB):
                    apply_ln(nb, hc2[nb], st2, h2[:, nb, :],
                             gb_t.get("g2bc"), gb_t.get("be2bc"))

        return unit, fin

    def pool_block(it):
        h2 = h2_t[it]
        gsb = pool_gsb.tile([P, 2], F32, tag="g", name=f"g_{it}")
        if pg_t[it] is None:
            pg = [ps_sm.tile([P, 1], F32, tag="h", name=f"pg_{it}_{kh}")
                  for kh in range(2)]
            for nb in range(NB):
                for kh in range(2):
                    nc.tensor.matmul(pg[kh][:],
                                     h2[:, nb, kh * P:(kh + 1) * P],
                                     ones_b[:], start=(nb == 0),
                                     stop=(nb == NB - 1))
        else:
            pg = pg_t[it]
        for kh in range(2):
            nc.scalar.mul(gsb[:, kh:kh + 1], pg[kh][:], 1.0 / N)

        for hd, (w_t, b_t, out_d) in enumerate(
                ((wa_t, ba_t, io["op"]), (wl_t, bl_t, io["ol"]))):
            po = ps_sm.tile([K, 1], F32, tag="h", name=f"po_{it}_{hd}")
            for kh in range(2):
                nc.tensor.matmul(po[:], w_t[kh][:], gsb[:, kh:kh + 1],
                                 start=(kh == 0), stop=(kh == 1))
            osb = pool_osb.tile([K, 1], F32, tag="o", name=f"o_{it}_{hd}")
            nc.scalar.activation(out=osb[:], in_=po[:],
                                 func=mybir.ActivationFunctionType.Identity,
                                 bias=b_t[:], scale=1.0)
            nc.sync.dma_start(out_d[it:it + 1, :], osb[:])

    # ---- software pipeline: dense phases interleave into the next agg
    # phase's c-loop; pool/head blocks trail their applies by one phase ----
    load(0, chunks=8)       # chunked so the first tiles land early
    emit_weight_dmas()
    if items == 1:
        l1_agg(0)
        unit, fin = make_l1_dense(0)
        for nb in range(NB):
            unit(nb)
        fin()
        l2_agg(0)
        unit, fin = make_l2_dense(0, fuse_pool=True)
        for nb in range(NB):
            unit(nb)
        fin()
        pool_block(0)
        es.close()
        return

    load(1)
    phases = [("l1", 0), ("l1", 1)]
    for it in range(items):
        phases.append(("l2", it))
        if it + 2 < items:
            phases.append(("l1", it + 2))

    ready = None            # (kind, it, unit, fin) pending dense phase
    pool_q = []             # items whose pool block is due next phase
    for kind, it in phases:
        pool_now, pool_q = pool_q, []
        cur, ready = ready, None
        unit = cur[2] if cur else None
        if kind == "l1":
            if it >= 2:
                load(it)
            l1_agg(it, unit)
        else:
            l2_agg(it, unit)
        if cur is not None:
            cur[3]()
            if cur[0] == "l2":
                pool_q.append(cur[1])
        for p in pool_now:
            pool_block(p)
        if kind == "l1":
            ready = ("l1", it) + make_l1_dense(it)
        else:
            ready = ("l2", it) + make_l2_dense(it, fuse_pool=(it == items - 1))

    # tail: the last item's dense phase has no agg left to hide in
    kind, itl, unit, fin = ready
    for nb in range(NB):
        unit(nb)
    for p in pool_q:
        pool_block(p)       # fills the finish_stats latency with PE work
    fin()
    pool_block(itl)

    es.close()


_CACHE = {}


def _get_nc(items, general):
    key = (items, general)
    if key not in _CACHE:
        nc = bacc.Bacc("TRN2", target_bir_lowering=False, debug=False,
                       num_devices=N_CORES)
        with tile.TileContext(nc) as tc:
            io = _declare_io(nc, items, general)
            _build_core(nc, tc, io, items, general)
        nc.compile()
        _CACHE[key] = nc
    return _CACHE[key]


def make_in_maps(A_hat, X, W1, b1, g1, beta1, W2, b2, g2, beta2,
                 Wa, ba, Wl, bl):
    """Host-side prep: shard over batch, transpose+cast A, fold gammas."""
    B = A_hat.shape[0]
    items = B // N_CORES
    general = bool(np.any(beta1 != 0) or np.any(beta2 != 0)
                   or np.any(g1 <= 0) or np.any(g2 <= 0))
    if general:
        w2f = np.asarray(W2, np.float32)
        waf = np.asarray(Wa, np.float32)
        wlf = np.asarray(Wl, np.float32)
    else:
        w2f = np.asarray(g1, np.float32)[:, None] * W2
        waf = (np.asarray(g2, np.float32)[:, None] * Wa).astype(np.float32)
        wlf = (np.asarray(g2, np.float32)[:, None] * Wl).astype(np.float32)
    w1f = np.asarray(W1, np.float32)
    w1e = np.concatenate([w1f, w1f.sum(1, keepdims=True)], 1)
    w2e = np.concatenate([w2f, w2f.sum(1, keepdims=True)], 1)
    shared = {
        "w1": w1e.astype(bf16),
        "w2": w2e.astype(bf16),
        "sb1": np.full((P, 1), np.float32(np.sum(np.asarray(b1, np.float32)))),
        "sb2": np.full((P, 1), np.float32(np.sum(np.asarray(b2, np.float32)))),
        "b1bc": np.ascontiguousarray(
            np.broadcast_to(np.asarray(b1, np.float32), (P, H))),
        "b2bc": np.ascontiguousarray(
            np.broadcast_to(np.asarray(b2, np.float32), (P, H))),
        "wa": waf, "wl": wlf,
        "ba": np.asarray(ba, np.float32).reshape(K, 1).copy(),
        "bl": np.asarray(bl, np.float32).reshape(K, 1).copy(),
        "ones": np.ones((P, 1), bf16),
    }
    if general:
        for nm, v in (("g1bc", g1), ("be1bc", beta1),
                      ("g2bc", g2), ("be2bc", beta2)):
            shared[nm] = np.ascontiguousarray(
                np.broadcast_to(np.asarray(v, np.float32), (P, H)))
    A_bf = np.asarray(A_hat, np.float32).astype(bf16)
    X_bf = np.asarray(X, np.float32).astype(bf16)
    in_maps = []
    for c in range(N_CORES):
        m = dict(shared)
        m["at4"] = np.ascontiguousarray(
            A_bf[c * items:(c + 1) * items].transpose(0, 2, 1))
        m["x4"] = np.ascontiguousarray(X_bf[c * items:(c + 1) * items])
        in_maps.append(m)
    return in_maps, items, general


def kernel(**inputs):
    in_maps, items, general = make_in_maps(**inputs)
    nc = _get_nc(items, general)
    res = run_bass_kernel_spmd(nc, in_maps, core_ids=list(range(N_CORES)))
    pred = np.concatenate([res.results[c]["op"] for c in range(N_CORES)], 0)
    logits = np.concatenate([res.results[c]["ol"] for c in range(N_CORES)], 0)
    return (np.asarray(pred, np.float32), np.asarray(logits, np.float32))


# revision 28
# speedup vs baseline: 1.7658x; 1.7658x over previous
"""TRN2 Bass kernel for nn_GCNBasic (2-layer GCN, B=32, N=2048, F=128, H=256).

Sharding: data-parallel over batch B across 8 NeuronCores (4 items/core);
small weights replicated.  A_hat is transposed and cast to bf16 on the HOST
(layout prep, halves HBM traffic); the device streams A^T tiles straight
into SBUF and runs pure matmul pipelines:

  (AX)^T[f,n]  = sum_mb  X[mb]-stationary   @ A^T[mb]   (c-outer, 4 psum
                                                         512-chunks live)
  H1pre[n,h]   = (AX)^T[:,nb]-stationary    @ W1
  H1           = relu(LN(H1pre + b1))        (bias-add Pool, sum DVE,
                                              sumsq+apply ACT, f32 stats)
  (AH)^T[hh,n] = sum_mb H1[mb,hh]-stationary @ A^T[mb]   (hh outer)
  H2pre[n,k]   = sum_hh (AH)^T[hh,nb]-stat.  @ diag(g1)W2
  H2           = relu(LN(H2pre + b2))
  g^T          = sum_nb H2[nb,kh]-stationary @ ones  (mean pool via PE)
  outputs      = diag(g2)Wa/Wl heads in fp32, biases added on ACT.

Items are software-pipelined (load it+2 / L1 of it+1 emitted between L2
and pool of it) so PE never waits on LayerNorm barriers or DMA refill.

gamma folds (diag(g1)@W2, diag(g2)@Wa/Wl) are exact because relu(g*z)=
g*relu(z) for g>0; beta==0 fast path (the problem's setup_inputs always
produces gamma=1, beta=0); a general gamma/beta path exists as a fallback.

Known TRN2 pitfalls worked around here: tensor_tensor_reduce crashes the
device; ACT/DVE writes into PSUM are unstable -> squares go to SBUF scratch.
"""

from contextlib import ExitStack

import numpy as np
import ml_dtypes

import concourse.bacc as bacc
import concourse.mybir as mybir
import concourse.tile as tile
from concourse.bass_utils import run_bass_kernel_spmd

F32 = mybir.dt.float32
BF16 = mybir.dt.bfloat16
bf16 = ml_dtypes.bfloat16

N = 2048
F = 128
H = 256
K = 64
P = 128
NB = N // P
NCH = N // 512
EPS = 1e-5
N_CORES = 8


def _declare_io(nc, items, general):
    io = {}
    io["at4"] = nc.dram_tensor("at4", [items, N, N], BF16, kind="ExternalInput")
    io["x4"] = nc.dram_tensor("x4", [items, N, F], BF16, kind="ExternalInput")
    # W1/W2 carry an extra trailing column holding W@1 so the dense matmul
    # also produces the LayerNorm row-sum (sans bias) as output column H.
    io["w1"] = nc.dram_tensor("w1", [F, H + 1], BF16, kind="ExternalInput")
    io["w2"] = nc.dram_tensor("w2", [H, H + 1], BF16, kind="ExternalInput")
    io["b1bc"] = nc.dram_tensor("b1bc", [P, H], F32, kind="ExternalInput")
    io["b2bc"] = nc.dram_tensor("b2bc", [P, H], F32, kind="ExternalInput")
    io["sb1"] = nc.dram_tensor("sb1", [P, 1], F32, kind="ExternalInput")
    io["sb2"] = nc.dram_tensor("sb2", [P, 1], F32, kind="ExternalInput")
    io["wa"] = nc.dram_tensor("wa", [H, K], F32, kind="ExternalInput")
    io["wl"] = nc.dram_tensor("wl", [H, K], F32, kind="ExternalInput")
    io["ba"] = nc.dram_tensor("ba", [K, 1], F32, kind="ExternalInput")
    io["bl"] = nc.dram_tensor("bl", [K, 1], F32, kind="ExternalInput")
    io["ones"] = nc.dram_tensor("ones", [P, 1], BF16, kind="ExternalInput")
    if general:
        io["g1bc"] = nc.dram_tensor("g1bc", [P, H], F32, kind="ExternalInput")
        io["be1bc"] = nc.dram_tensor("be1bc", [P, H], F32, kind="ExternalInput")
        io["g2bc"] = nc.dram_tensor("g2bc", [P, H], F32, kind="ExternalInput")
        io["be2bc"] = nc.dram_tensor("be2bc", [P, H], F32, kind="ExternalInput")
    io["op"] = nc.dram_tensor("op", [items, K], F32, kind="ExternalOutput")
    io["ol"] = nc.dram_tensor("ol", [items, K], F32, kind="ExternalOutput")
    return io


def _build_core(nc, tc, io, items, general):
    at4, x4 = io["at4"], io["x4"]
    es = ExitStack()

    consts = es.enter_context(tc.tile_pool(name="consts", bufs=1))
    wts = es.enter_context(tc.tile_pool(name="wts", bufs=1))
    pool_at = es.enter_context(tc.tile_pool(name="at", bufs=2 * NB))
    pool_xb = es.enter_context(tc.tile_pool(name="xb", bufs=2))
    pool_axT = es.enter_context(tc.tile_pool(name="axT", bufs=2))
    pool_h1 = es.enter_context(tc.tile_pool(name="h1", bufs=2))
    pool_ahT = es.enter_context(tc.tile_pool(name="ahT", bufs=2))
    pool_h2 = es.enter_context(tc.tile_pool(name="h2", bufs=1))
    pool_hc = es.enter_context(tc.tile_pool(name="hc", bufs=NB))
    pool_sq = es.enter_context(tc.tile_pool(name="sq", bufs=2))
    pool_st = es.enter_context(tc.tile_pool(name="st", bufs=4))
    pool_gsb = es.enter_context(tc.tile_pool(name="gsb", bufs=4))
    pool_osb = es.enter_context(tc.tile_pool(name="osb", bufs=4))

    ps_big = es.enter_context(tc.tile_pool(name="ps_big", bufs=6, space="PSUM"))
    ps_h = es.enter_context(tc.tile_pool(name="ps_h", bufs=2, space="PSUM"))
    ps_sm = ps_h  # pg/po share the ps_h banks (never live at the same time)

    eps_t = consts.tile([P, 1], F32)
    nc.vector.memset(eps_t[:], EPS)
    ones_b = consts.tile([P, 1], BF16)
    w1_t = wts.tile([P, H + 1], BF16)
    w2_t = [wts.tile([P, H + 1], BF16, tag=f"w2_{hh}", name=f"w2_{hh}")
            for hh in range(2)]
    b1_t = wts.tile([P, H], F32)
    b2_t = wts.tile([P, H], F32)
    sb1_t = wts.tile([P, 1], F32)
    sb2_t = wts.tile([P, 1], F32)
    wa_t = [wts.tile([P, K], F32, tag=f"wa_{hh}", name=f"wa_{hh}")
            for hh in range(2)]
    wl_t = [wts.tile([P, K], F32, tag=f"wl_{hh}", name=f"wl_{hh}")
            for hh in range(2)]
    ba_t = wts.tile([K, 1], F32)
    bl_t = wts.tile([K, 1], F32)
    gb_t = {}
    if general:
        for nm in ("g1bc", "be1bc", "g2bc", "be2bc"):
            gb_t[nm] = wts.tile([P, H], F32, tag=nm, name=nm)

    def emit_weight_dmas():
        nc.sync.dma_start(ones_b[:], io["ones"][:])
        nc.sync.dma_start(w1_t[:], io["w1"][:])
        for hh in range(2):
            nc.sync.dma_start(w2_t[hh][:], io["w2"][hh * P:(hh + 1) * P, :])
        nc.sync.dma_start(b1_t[:], io["b1bc"][:])
        nc.sync.dma_start(b2_t[:], io["b2bc"][:])
        nc.sync.dma_start(sb1_t[:], io["sb1"][:])
        nc.sync.dma_start(sb2_t[:], io["sb2"][:])
        for hh in range(2):
            nc.sync.dma_start(wa_t[hh][:], io["wa"][hh * P:(hh + 1) * P, :])
            nc.sync.dma_start(wl_t[hh][:], io["wl"][hh * P:(hh + 1) * P, :])
        nc.sync.dma_start(ba_t[:], io["ba"][:])
        nc.sync.dma_start(bl_t[:], io["bl"][:])
        for nm, t in gb_t.items():
            nc.sync.dma_start(t[:], io[nm][:])

    inv_h = 1.0 / H

    # per-item live tiles (indexed by item)
    at_t = [None] * items
    xb_t = [None] * items
    axT_t = [None] * items
    h1_t = [None] * items
    ahT_t = [None] * items
    h2_t = [None] * items
    hc1_t = [None] * items
    hc2_t = [None] * items
    st1_t = [None] * items
    st2_t = [None] * items

    def load(it, chunks=1):
        xb = pool_xb.tile([P, NB, F], BF16, tag="xb", name=f"xb_{it}")
        nc.sync.dma_start(xb[:], x4[it].rearrange("(c p) f -> p c f", p=P))
        xb_t[it] = xb
        ats = [pool_at.tile([P, N], BF16, tag="at", name=f"at_{it}_{c}")
               for c in range(NB)]
        cw = N // chunks
        for c in range(NB):
            for k in range(chunks):
                nc.sync.dma_start(
                    ats[c][:, k * cw:(k + 1) * cw],
                    at4[it, c * P:(c + 1) * P, k * cw:(k + 1) * cw])
        at_t[it] = ats

    def l1_agg(it, unit=None):
        # one pending dense unit (PE matmul + LN stats) dripped per c
        # iteration so the LN chain paces alongside pure agg matmuls
        at, xb = at_t[it], xb_t[it]
        pb = [ps_big.tile([P, 512], F32, tag="big", name=f"ax_{it}_{j}")
              for j in range(NCH)]
        for c in range(NB):
            for j in range(NCH):
                nc.tensor.matmul(pb[j][:], xb[:, c, :],
                                 at[c][:, j * 512:(j + 1) * 512],
                                 start=(c == 0), stop=(c == NB - 1))
            if unit is not None:
                unit(c)
        axT = pool_axT.tile([P, N], BF16, tag="axT", name=f"axT_{it}")
        for j in range(NCH):
            nc.vector.tensor_copy(axT[:, j * 512:(j + 1) * 512], pb[j][:])
        axT_t[it] = axT

    def ln_stats(nb, ph, b_t, sb_t, st, hc, sfx):
        # bias add on DVE (PSUM->SBUF); row-sum comes from the matmul's
        # extra column H (+ bias total from sb); sumsq on Pool (one
        # scalar_tensor_tensor with accum_out -- keeps ACT free for applies)
        nc.vector.tensor_tensor(out=hc[:], in0=ph[:, 0:H], in1=b_t[:],
                                op=mybir.AluOpType.add)
        nc.vector.tensor_tensor(out=st[:, 0, nb:nb + 1], in0=ph[:, H:H + 1],
                                in1=sb_t[:], op=mybir.AluOpType.add)
        sq = pool_sq.tile([P, H], F32, tag="sq", name=f"sq_{sfx}")
        nc.scalar.activation(
            out=sq[:], in_=hc[:], func=mybir.ActivationFunctionType.Square,
            accum_out=st[:, 1, nb:nb + 1])

    def finish_stats(st):
        s = st
        nc.vector.tensor_scalar(out=s[:, 2, :], in0=s[:, 0, :],
                                scalar1=-inv_h, scalar2=None,
                                op0=mybir.AluOpType.mult)          # -mu
        nc.vector.tensor_tensor(out=s[:, 3, :], in0=s[:, 2, :], in1=s[:, 2, :],
                                op=mybir.AluOpType.mult)           # mu^2
        nc.vector.tensor_scalar(out=s[:, 4, :], in0=s[:, 1, :],
                                scalar1=inv_h, scalar2=None,
                                op0=mybir.AluOpType.mult)          # E[x^2]
        nc.vector.tensor_tensor(out=s[:, 4, :], in0=s[:, 4, :], in1=s[:, 3, :],
                                op=mybir.AluOpType.subtract)       # var
        nc.scalar.activation(out=s[:, 5, :], in_=s[:, 4, :],
                             func=mybir.ActivationFunctionType.Sqrt,
                             bias=eps_t[:], scale=1.0)             # sd
        nc.vector.reciprocal(out=s[:, 6, :], in_=s[:, 5, :])       # 1/sd
        nc.vector.tensor_tensor(out=s[:, 7, :], in0=s[:, 2, :], in1=s[:, 6, :],
                                op=mybir.AluOpType.mult)           # -mu/sd

    def apply_ln(nb, hc, st, h_out, g_bc, be_bc, use_act=True):
        if not general:
            if use_act:
                nc.scalar.activation(out=h_out, in_=hc[:],
                                     func=mybir.ActivationFunctionType.Relu,
                                     bias=st[:, 7, nb:nb + 1],
                                     scale=st[:, 6, nb:nb + 1])
            else:
                # Pool two-op apply keeps ACT free for the squares
                nc.gpsimd.tensor_scalar(out=hc[:], in0=hc[:],
                                        scalar1=st[:, 6, nb:nb + 1],
                                        scalar2=st[:, 7, nb:nb + 1],
                                        op0=mybir.AluOpType.mult,
                                        op1=mybir.AluOpType.add)
                nc.gpsimd.tensor_scalar_max(h_out, hc[:], 0.0)
        else:
            nc.scalar.activation(out=hc[:], in_=hc[:],
                                 func=mybir.ActivationFunctionType.Identity,
                                 bias=st[:, 7, nb:nb + 1],
                                 scale=st[:, 6, nb:nb + 1])
            nc.gpsimd.tensor_tensor(out=hc[:], in0=hc[:], in1=g_bc[:],
                                    op=mybir.AluOpType.mult)
            nc.vector.tensor_tensor(out=hc[:], in0=hc[:], in1=be_bc[:],
                                    op=mybir.AluOpType.add)
            nc.scalar.activation(out=h_out, in_=hc[:],
                                 func=mybir.ActivationFunctionType.Relu)

    def l2_agg(it, unit=None):
        at, h1 = at_t[it], h1_t[it]
        ahT = [pool_ahT.tile([P, N], BF16, tag="ahT", name=f"ahT_{it}_{hh}")
               for hh in range(2)]
        for hh in range(2):
            pb = [ps_big.tile([P, 512], F32, tag="big",
                              name=f"ah_{it}_{hh}_{j}") for j in range(NCH)]
            for c in range(NB):
                for j in range(NCH):
                    nc.tensor.matmul(pb[j][:], h1[:, c, hh * P:(hh + 1) * P],
                                     at[c][:, j * 512:(j + 1) * 512],
                                     start=(c == 0), stop=(c == NB - 1))
                if unit is not None and hh == 0:
                    unit(c)
            for j in range(NCH):
                nc.vector.tensor_copy(
                    ahT[hh][:, j * 512:(j + 1) * 512], pb[j][:])
        ahT_t[it] = ahT

    pg_t = [None] * items

    def make_l1_dense(it):
        axT = axT_t[it]
        st1 = pool_st.tile([P, 8, NB], F32, tag="st", name=f"st1_{it}")
        h1 = pool_h1.tile([P, NB, H], BF16, tag="h1", name=f"h1_{it}")
        hc1 = []
        st1_t[it], h1_t[it] = st1, h1

        def unit(nb):
            ph = ps_h.tile([P, H + 1], F32, tag="h", name=f"p1_{it}_{nb}")
            nc.tensor.matmul(ph[:], axT[:, nb * P:(nb + 1) * P], w1_t[:],
                             start=True, stop=True)
            hc = pool_hc.tile([P, H], F32, tag="hc", name=f"hc1_{it}_{nb}")
            ln_stats(nb, ph, b1_t, sb1_t, st1, hc, f"1_{it}_{nb}")
            hc1.append(hc)

        def fin():
            finish_stats(st1)
            for nb in range(NB):
                apply_ln(nb, hc1[nb], st1, h1[:, nb, :],
                         gb_t.get("g1bc"), gb_t.get("be1bc"))

        return unit, fin

    def make_l2_dense(it, fuse_pool=False):
        ahT = ahT_t[it]
        st2 = pool_st.tile([P, 8, NB], F32, tag="st", name=f"st2_{it}")
        h2 = pool_h2.tile([P, NB, H], BF16, tag="h2", name=f"h2_{it}")
        hc2 = []
        st2_t[it], h2_t[it] = st2, h2

        def unit(nb):
            ph = ps_h.tile([P, H + 1], F32, tag="h", name=f"p2_{it}_{nb}")
            for hh in range(2):
                nc.tensor.matmul(ph[:], ahT[hh][:, nb * P:(nb + 1) * P],
                                 w2_t[hh][:], start=(hh == 0), stop=(hh == 1))
            hc = pool_hc.tile([P, H], F32, tag="hc", name=f"hc2_{it}_{nb}")
            ln_stats(nb, ph, b2_t, sb2_t, st2, hc, f"2_{it}_{nb}")
            hc2.append(hc)

        def fin():
            finish_stats(st2)
            if fuse_pool:
                # last item: interleave mean-pool accumulation with the
                # applies so the PE doesn't sit out the whole LN2 tail;
                # applies alternate ACT / Pool to halve the serial chain
                pg = [ps_sm.tile([P, 1], F32, tag="h", name=f"pg_{it}_{kh}")
                      for kh in range(2)]
                for nb in range(NB):
                    apply_ln(nb, hc2[nb], st2, h2[:, nb, :],
                             gb_t.get("g2bc"), gb_t.get("be2bc"))
                    for kh in range(2):
                        nc.tensor.matmul(pg[kh][:],
                                         h2[:, nb, kh * P:(kh + 1) * P],
                                         ones_b[:], start=(nb == 0),
                                         stop=(nb == NB - 1))
                pg_t[it] = pg
            else:
                for nb in range(NB):
                    apply_ln(nb, hc2[nb], st2, h2[:, nb, :],
                             gb_t.get("g2bc"), gb_t.get("be2bc"))

        return unit, fin

    def pool_block(it):
        h2 = h2_t[it]
        gsb = pool_gsb.tile([P, 2], F32, tag="g", name=f"g_{it}")
        if pg_t[it] is None:
            pg = [ps_sm.tile([P, 1], F32, tag="h", name=f"pg_{it}_{kh}")
                  for kh in range(2)]
            for nb in range(NB):
                for kh in range(2):
                    nc.tensor.matmul(pg[kh][:],
                                     h2[:, nb, kh * P:(kh + 1) * P],
                                     ones_b[:], start=(nb == 0),
                                     stop=(nb == NB - 1))
        else:
            pg = pg_t[it]
        for kh in range(2):
            nc.scalar.mul(gsb[:, kh:kh + 1], pg[kh][:], 1.0 / N)

        for hd, (w_t, b_t, out_d) in enumerate(
                ((wa_t, ba_t, io["op"]), (wl_t, bl_t, io["ol"]))):
            po = ps_sm.tile([K, 1], F32, tag="h", name=f"po_{it}_{hd}")
            for kh in range(2):
                nc.tensor.matmul(po[:], w_t[kh][:], gsb[:, kh:kh + 1],
                                 start=(kh == 0), stop=(kh == 1))
            osb = pool_osb.tile([K, 1], F32, tag="o", name=f"o_{it}_{hd}")
            nc.scalar.activation(out=osb[:], in_=po[:],
                                 func=mybir.ActivationFunctionType.Identity,
                                 bias=b_t[:], scale=1.0)
            nc.sync.dma_start(out_d[it:it + 1, :], osb[:])

    # ---- software pipeline: dense phases interleave into the next agg
    # phase's c-loop; pool/head blocks trail their applies by one phase ----
    load(0, chunks=8)       # chunked so the first tiles land early
    emit_weight_dmas()
    if items == 1:
        l1_agg(0)
        unit, fin = make_l1_dense(0)
        for nb in range(NB):
            unit(nb)
        fin()
        l2_agg(0)
        unit, fin = make_l2_dense(0, fuse_pool=True)
        for nb in range(NB):
            unit(nb)
        fin()
        pool_block(0)
        es.close()
        return

    load(1)
    phases = [("l1", 0), ("l1", 1)]
    for it in range(items):
        phases.append(("l2", it))
        if it + 2 < items:
            phases.append(("l1", it + 2))

    ready = None            # (kind, it, unit, fin) pending dense phase
    pool_q = []             # items whose pool block is due next phase
    for kind, it in phases:
        pool_now, pool_q = pool_q, []
        cur, ready = ready, None
        unit = cur[2] if cur else None
        if kind == "l1":
            if it >= 2:
                load(it)
            l1_agg(it, unit)
        else:
            l2_agg(it, unit)
        if cur is not None:
            cur[3]()
            if cur[0] == "l2":
                pool_q.append(cur[1])
        for p in pool_now:
            pool_block(p)
        if kind == "l1":
            ready = ("l1", it) + make_l1_dense(it)
        else:
            ready = ("l2", it) + make_l2_dense(it, fuse_pool=(it == items - 1))

    # tail: the last item's dense phase has no agg left to hide in
    kind, itl, unit, fin = ready
    for nb in range(NB):
        unit(nb)
    for p in pool_q:
        pool_block(p)       # fills the finish_stats latency with PE work
    fin()
    pool_block(itl)

    es.close()


_CACHE = {}


def _get_nc(items, general):
    key = (items, general)
    if key not in _CACHE:
        nc = bacc.Bacc("TRN2", target_bir_lowering=False, debug=False,
                       num_devices=N_CORES)
        with tile.TileContext(nc) as tc:
            io = _declare_io(nc, items, general)
            _build_core(nc, tc, io, items, general)
        nc.compile()
        _CACHE[key] = nc
    return _CACHE[key]


def make_in_maps(A_hat, X, W1, b1, g1, beta1, W2, b2, g2, beta2,
                 Wa, ba, Wl, bl):
    """Host-side prep: shard over batch, transpose+cast A, fold gammas."""
    B = A_hat.shape[0]
    items = B // N_CORES
    general = bool(np.any(beta1 != 0) or np.any(beta2 != 0)
                   or np.any(g1 <= 0) or np.any(g2 <= 0))
    if general:
        w2f = np.asarray(W2, np.float32)
        waf = np.asarray(Wa, np.float32)
        wlf = np.asarray(Wl, np.float32)
    else:
        w2f = np.asarray(g1, np.float32)[:, None] * W2
        waf = (np.asarray(g2, np.float32)[:, None] * Wa).astype(np.float32)
        wlf = (np.asarray(g2, np.float32)[:, None] * Wl).astype(np.float32)
    w1f = np.asarray(W1, np.float32)
    w1e = np.concatenate([w1f, w1f.sum(1, keepdims=True)], 1)
    w2e = np.concatenate([w2f, w2f.sum(1, keepdims=True)], 1)
    shared = {
        "w1": w1e.astype(bf16),
        "w2": w2e.astype(bf16),
        "sb1": np.full((P, 1), np.float32(np.sum(np.asarray(b1, np.float32)))),
        "sb2": np.full((P, 1), np.float32(np.sum(np.asarray(b2, np.float32)))),
        "b1bc": np.ascontiguousarray(
            np.broadcast_to(np.asarray(b1, np.float32), (P, H))),
        "b2bc": np.ascontiguousarray(
            np.broadcast_to(np.asarray(b2, np.float32), (P, H))),
        "wa": waf, "wl": wlf,
        "ba": np.asarray(ba, np.float32).reshape(K, 1).copy(),
        "bl": np.asarray(bl, np.float32).reshape(K, 1).copy(),
        "ones": np.ones((P, 1), bf16),
    }
    if general:
        for nm, v in (("g1bc", g1), ("be1bc", beta1),
                      ("g2bc", g2), ("be2bc", beta2)):
            shared[nm] = np.ascontiguousarray(
                np.broadcast_to(np.asarray(v, np.float32), (P, H)))
    A_bf = np.asarray(A_hat, np.float32).astype(bf16)
    X_bf = np.asarray(X, np.float32).astype(bf16)
    in_maps = []
    for c in range(N_CORES):
        m = dict(shared)
        m["at4"] = np.ascontiguousarray(
            A_bf[c * items:(c + 1) * items].transpose(0, 2, 1))
        m["x4"] = np.ascontiguousarray(X_bf[c * items:(c + 1) * items])
        in_maps.append(m)
    return in_maps, items, general


def kernel(**inputs):
    in_maps, items, general = make_in_maps(**inputs)
    nc = _get_nc(items, general)
    res = run_bass_kernel_spmd(nc, in_maps, core_ids=list(range(N_CORES)))
    pred = np.concatenate([res.results[c]["op"] for c in range(N_CORES)], 0)
    logits = np.concatenate([res.results[c]["ol"] for c in range(N_CORES)], 0)
    return (np.asarray(pred, np.float32), np.asarray(logits, np.float32))


# revision 29
# speedup vs baseline: 2.3747x; 1.3448x over previous
"""TRN2 Bass kernel for nn_GCNBasic (2-layer GCN, B=32, N=2048, F=128, H=256).

Sharding: data-parallel over batch B across 8 NeuronCores (4 items/core);
small weights replicated.  A_hat is transposed and cast to bf16 on the HOST
(layout prep, halves HBM traffic); the device streams A^T tiles straight
into SBUF and runs pure matmul pipelines:

  (AX)^T[f,n]  = sum_mb  X[mb]-stationary   @ A^T[mb]   (c-outer, 4 psum
                                                         512-chunks live)
  H1pre[n,h]   = (AX)^T[:,nb]-stationary    @ W1
  H1           = relu(LN(H1pre + b1))        (bias-add Pool, sum DVE,
                                              sumsq+apply ACT, f32 stats)
  (AH)^T[hh,n] = sum_mb H1[mb,hh]-stationary @ A^T[mb]   (hh outer)
  H2pre[n,k]   = sum_hh (AH)^T[hh,nb]-stat.  @ diag(g1)W2
  H2           = relu(LN(H2pre + b2))
  g^T          = sum_nb H2[nb,kh]-stationary @ ones  (mean pool via PE)
  outputs      = diag(g2)Wa/Wl heads in fp32, biases added on ACT.

Items are software-pipelined (load it+2 / L1 of it+1 emitted between L2
and pool of it) so PE never waits on LayerNorm barriers or DMA refill.

gamma folds (diag(g1)@W2, diag(g2)@Wa/Wl) are exact because relu(g*z)=
g*relu(z) for g>0; beta==0 fast path (the problem's setup_inputs always
produces gamma=1, beta=0); a general gamma/beta path exists as a fallback.

Known TRN2 pitfalls worked around here: tensor_tensor_reduce crashes the
device; ACT/DVE writes into PSUM are unstable -> squares go to SBUF scratch.
"""

from contextlib import ExitStack

import numpy as np
import ml_dtypes

import concourse.bacc as bacc
import concourse.mybir as mybir
import concourse.tile as tile
from concourse.bass_utils import run_bass_kernel_spmd

F32 = mybir.dt.float32
BF16 = mybir.dt.bfloat16
bf16 = ml_dtypes.bfloat16

N = 2048
F = 128
H = 256
K = 64
P = 128
NB = N // P
NCH = N // 512
EPS = 1e-5
N_CORES = 8


def _declare_io(nc, items, general):
    io = {}
    io["at4"] = nc.dram_tensor("at4", [items, N, N], BF16, kind="ExternalInput")
    io["x4"] = nc.dram_tensor("x4", [items, N, F], BF16, kind="ExternalInput")
    # W1/W2 carry an extra trailing column holding W@1 so the dense matmul
    # also produces the LayerNorm row-sum (sans bias) as output column H.
    io["w1"] = nc.dram_tensor("w1", [F, H + 1], BF16, kind="ExternalInput")
    io["w2"] = nc.dram_tensor("w2", [H, H + 1], BF16, kind="ExternalInput")
    io["b1bc"] = nc.dram_tensor("b1bc", [P, H], F32, kind="ExternalInput")
    io["b2bc"] = nc.dram_tensor("b2bc", [P, H], F32, kind="ExternalInput")
    io["sb1"] = nc.dram_tensor("sb1", [P, 1], F32, kind="ExternalInput")
    io["sb2"] = nc.dram_tensor("sb2", [P, 1], F32, kind="ExternalInput")
    io["wa"] = nc.dram_tensor("wa", [H, K], F32, kind="ExternalInput")
    io["wl"] = nc.dram_tensor("wl", [H, K], F32, kind="ExternalInput")
    io["ba"] = nc.dram_tensor("ba", [K, 1], F32, kind="ExternalInput")
    io["bl"] = nc.dram_tensor("bl", [K, 1], F32, kind="ExternalInput")
    io["ones"] = nc.dram_tensor("ones", [P, 1], BF16, kind="ExternalInput")
    if general:
        io["g1bc"] = nc.dram_tensor("g1bc", [P, H], F32, kind="ExternalInput")
        io["be1bc"] = nc.dram_tensor("be1bc", [P, H], F32, kind="ExternalInput")
        io["g2bc"] = nc.dram_tensor("g2bc", [P, H], F32, kind="ExternalInput")
        io["be2bc"] = nc.dram_tensor("be2bc", [P, H], F32, kind="ExternalInput")
    io["op"] = nc.dram_tensor("op", [items, K], F32, kind="ExternalOutput")
    io["ol"] = nc.dram_tensor("ol", [items, K], F32, kind="ExternalOutput")
    return io


def _build_core(nc, tc, io, items, general):
    at4, x4 = io["at4"], io["x4"]
    es = ExitStack()

    consts = es.enter_context(tc.tile_pool(name="consts", bufs=1))
    wts = es.enter_context(tc.tile_pool(name="wts", bufs=1))
    pool_at = es.enter_context(tc.tile_pool(name="at", bufs=2 * NB))
    pool_xb = es.enter_context(tc.tile_pool(name="xb", bufs=2))
    pool_axT = es.enter_context(tc.tile_pool(name="axT", bufs=2))
    pool_h1 = es.enter_context(tc.tile_pool(name="h1", bufs=2))
    pool_ahT = es.enter_context(tc.tile_pool(name="ahT", bufs=2))
    pool_h2 = es.enter_context(tc.tile_pool(name="h2", bufs=1))
    pool_hc = es.enter_context(tc.tile_pool(name="hc", bufs=NB))
    pool_sq = es.enter_context(tc.tile_pool(name="sq", bufs=2))
    pool_st = es.enter_context(tc.tile_pool(name="st", bufs=4))
    pool_gsb = es.enter_context(tc.tile_pool(name="gsb", bufs=4))
    pool_osb = es.enter_context(tc.tile_pool(name="osb", bufs=4))

    ps_big = es.enter_context(tc.tile_pool(name="ps_big", bufs=6, space="PSUM"))
    ps_h = es.enter_context(tc.tile_pool(name="ps_h", bufs=2, space="PSUM"))
    ps_sm = ps_h  # pg/po share the ps_h banks (never live at the same time)

    eps_t = consts.tile([P, 1], F32)
    nc.vector.memset(eps_t[:], EPS)
    ones_b = consts.tile([P, 1], BF16)
    w1_t = wts.tile([P, H + 1], BF16)
    w2_t = [wts.tile([P, H + 1], BF16, tag=f"w2_{hh}", name=f"w2_{hh}")
            for hh in range(2)]
    b1_t = wts.tile([P, H], F32)
    b2_t = wts.tile([P, H], F32)
    sb1_t = wts.tile([P, 1], F32)
    sb2_t = wts.tile([P, 1], F32)
    wa_t = [wts.tile([P, K], F32, tag=f"wa_{hh}", name=f"wa_{hh}")
            for hh in range(2)]
    wl_t = [wts.tile([P, K], F32, tag=f"wl_{hh}", name=f"wl_{hh}")
            for hh in range(2)]
    ba_t = wts.tile([K, 1], F32)
    bl_t = wts.tile([K, 1], F32)
    gb_t = {}
    if general:
        for nm in ("g1bc", "be1bc", "g2bc", "be2bc"):
            gb_t[nm] = wts.tile([P, H], F32, tag=nm, name=nm)

    def emit_weight_dmas():
        nc.sync.dma_start(ones_b[:], io["ones"][:])
        nc.sync.dma_start(w1_t[:], io["w1"][:])
        for hh in range(2):
            nc.sync.dma_start(w2_t[hh][:], io["w2"][hh * P:(hh + 1) * P, :])
        nc.sync.dma_start(b1_t[:], io["b1bc"][:])
        nc.sync.dma_start(b2_t[:], io["b2bc"][:])
        nc.sync.dma_start(sb1_t[:], io["sb1"][:])
        nc.sync.dma_start(sb2_t[:], io["sb2"][:])
        for hh in range(2):
            nc.sync.dma_start(wa_t[hh][:], io["wa"][hh * P:(hh + 1) * P, :])
            nc.sync.dma_start(wl_t[hh][:], io["wl"][hh * P:(hh + 1) * P, :])
        nc.sync.dma_start(ba_t[:], io["ba"][:])
        nc.sync.dma_start(bl_t[:], io["bl"][:])
        for nm, t in gb_t.items():
            nc.sync.dma_start(t[:], io[nm][:])

    inv_h = 1.0 / H

    # per-item live tiles (indexed by item)
    at_t = [None] * items
    xb_t = [None] * items
    axT_t = [None] * items
    h1_t = [None] * items
    ahT_t = [None] * items
    h2_t = [None] * items
    hc1_t = [None] * items
    hc2_t = [None] * items
    st1_t = [None] * items
    st2_t = [None] * items

    def load(it, chunks=1):
        xb = pool_xb.tile([P, NB, F], BF16, tag="xb", name=f"xb_{it}")
        nc.sync.dma_start(xb[:], x4[it].rearrange("(c p) f -> p c f", p=P))
        xb_t[it] = xb
        ats = [pool_at.tile([P, N], BF16, tag="at", name=f"at_{it}_{c}")
               for c in range(NB)]
        cw = N // chunks
        for c in range(NB):
            for k in range(chunks):
                nc.sync.dma_start(
                    ats[c][:, k * cw:(k + 1) * cw],
                    at4[it, c * P:(c + 1) * P, k * cw:(k + 1) * cw])
        at_t[it] = ats

    def l1_agg(it, unit=None):
        # one pending dense unit (PE matmul + LN stats) dripped per c
        # iteration so the LN chain paces alongside pure agg matmuls
        at, xb = at_t[it], xb_t[it]
        pb = [ps_big.tile([P, 512], F32, tag="big", name=f"ax_{it}_{j}")
              for j in range(NCH)]
        for c in range(NB):
            for j in range(NCH):
                nc.tensor.matmul(pb[j][:], xb[:, c, :],
                                 at[c][:, j * 512:(j + 1) * 512],
                                 start=(c == 0), stop=(c == NB - 1))
            if unit is not None:
                unit(c)
        axT = pool_axT.tile([P, N], BF16, tag="axT", name=f"axT_{it}")
        for j in range(NCH):
            if j % 2 == 0:
                nc.scalar.copy(axT[:, j * 512:(j + 1) * 512], pb[j][:])
            else:
                nc.vector.tensor_copy(axT[:, j * 512:(j + 1) * 512], pb[j][:])
        axT_t[it] = axT

    def ln_stats(nb, ph, b_t, sb_t, st, hc, sfx):
        # bias add on DVE (PSUM->SBUF); row-sum comes from the matmul's
        # extra column H (+ bias total from sb); sumsq on Pool (one
        # scalar_tensor_tensor with accum_out -- keeps ACT free for applies)
        nc.vector.tensor_tensor(out=hc[:], in0=ph[:, 0:H], in1=b_t[:],
                                op=mybir.AluOpType.add)
        nc.vector.tensor_tensor(out=st[:, 0, nb:nb + 1], in0=ph[:, H:H + 1],
                                in1=sb_t[:], op=mybir.AluOpType.add)
        sq = pool_sq.tile([P, H], F32, tag="sq", name=f"sq_{sfx}")
        nc.scalar.activation(
            out=sq[:], in_=hc[:], func=mybir.ActivationFunctionType.Square,
            accum_out=st[:, 1, nb:nb + 1])

    def finish_stats(st):
        s = st
        nc.vector.tensor_scalar(out=s[:, 2, :], in0=s[:, 0, :],
                                scalar1=-inv_h, scalar2=None,
                                op0=mybir.AluOpType.mult)          # -mu
        nc.vector.tensor_tensor(out=s[:, 3, :], in0=s[:, 2, :], in1=s[:, 2, :],
                                op=mybir.AluOpType.mult)           # mu^2
        nc.vector.tensor_scalar(out=s[:, 4, :], in0=s[:, 1, :],
                                scalar1=inv_h, scalar2=None,
                                op0=mybir.AluOpType.mult)          # E[x^2]
        nc.vector.tensor_tensor(out=s[:, 4, :], in0=s[:, 4, :], in1=s[:, 3, :],
                                op=mybir.AluOpType.subtract)       # var
        nc.scalar.activation(out=s[:, 5, :], in_=s[:, 4, :],
                             func=mybir.ActivationFunctionType.Sqrt,
                             bias=eps_t[:], scale=1.0)             # sd
        nc.vector.reciprocal(out=s[:, 6, :], in_=s[:, 5, :])       # 1/sd
        nc.vector.tensor_tensor(out=s[:, 7, :], in0=s[:, 2, :], in1=s[:, 6, :],
                                op=mybir.AluOpType.mult)           # -mu/sd

    def apply_ln(nb, hc, st, h_out, g_bc, be_bc, use_act=True):
        if not general:
            if use_act:
                nc.scalar.activation(out=h_out, in_=hc[:],
                                     func=mybir.ActivationFunctionType.Relu,
                                     bias=st[:, 7, nb:nb + 1],
                                     scale=st[:, 6, nb:nb + 1])
            else:
                # Pool two-op apply keeps ACT free for the squares
                nc.gpsimd.tensor_scalar(out=hc[:], in0=hc[:],
                                        scalar1=st[:, 6, nb:nb + 1],
                                        scalar2=st[:, 7, nb:nb + 1],
                                        op0=mybir.AluOpType.mult,
                                        op1=mybir.AluOpType.add)
                nc.gpsimd.tensor_scalar_max(h_out, hc[:], 0.0)
        else:
            nc.scalar.activation(out=hc[:], in_=hc[:],
                                 func=mybir.ActivationFunctionType.Identity,
                                 bias=st[:, 7, nb:nb + 1],
                                 scale=st[:, 6, nb:nb + 1])
            nc.gpsimd.tensor_tensor(out=hc[:], in0=hc[:], in1=g_bc[:],
                                    op=mybir.AluOpType.mult)
            nc.vector.tensor_tensor(out=hc[:], in0=hc[:], in1=be_bc[:],
                                    op=mybir.AluOpType.add)
            nc.scalar.activation(out=h_out, in_=hc[:],
                                 func=mybir.ActivationFunctionType.Relu)

    def l2_agg(it, unit=None):
        at, h1 = at_t[it], h1_t[it]
        ahT = [pool_ahT.tile([P, N], BF16, tag="ahT", name=f"ahT_{it}_{hh}")
               for hh in range(2)]
        for hh in range(2):
            pb = [ps_big.tile([P, 512], F32, tag="big",
                              name=f"ah_{it}_{hh}_{j}") for j in range(NCH)]
            for c in range(NB):
                for j in range(NCH):
                    nc.tensor.matmul(pb[j][:], h1[:, c, hh * P:(hh + 1) * P],
                                     at[c][:, j * 512:(j + 1) * 512],
                                     start=(c == 0), stop=(c == NB - 1))
                if unit is not None and hh == 0:
                    unit(c)
            for j in range(NCH):
                if j % 2 == 0:
                    nc.scalar.copy(ahT[hh][:, j * 512:(j + 1) * 512], pb[j][:])
                else:
                    nc.vector.tensor_copy(
                        ahT[hh][:, j * 512:(j + 1) * 512], pb[j][:])
        ahT_t[it] = ahT

    pg_t = [None] * items

    def make_l1_dense(it):
        axT = axT_t[it]
        st1 = pool_st.tile([P, 8, NB], F32, tag="st", name=f"st1_{it}")
        h1 = pool_h1.tile([P, NB, H], BF16, tag="h1", name=f"h1_{it}")
        hc1 = []
        st1_t[it], h1_t[it] = st1, h1

        def unit(nb):
            ph = ps_h.tile([P, H + 1], F32, tag="h", name=f"p1_{it}_{nb}")
            nc.tensor.matmul(ph[:], axT[:, nb * P:(nb + 1) * P], w1_t[:],
                             start=True, stop=True)
            hc = pool_hc.tile([P, H], F32, tag="hc", name=f"hc1_{it}_{nb}")
            ln_stats(nb, ph, b1_t, sb1_t, st1, hc, f"1_{it}_{nb}")
            hc1.append(hc)

        def fin():
            finish_stats(st1)
            for nb in range(NB):
                apply_ln(nb, hc1[nb], st1, h1[:, nb, :],
                         gb_t.get("g1bc"), gb_t.get("be1bc"))

        return unit, fin

    def make_l2_dense(it, fuse_pool=False):
        ahT = ahT_t[it]
        st2 = pool_st.tile([P, 8, NB], F32, tag="st", name=f"st2_{it}")
        h2 = pool_h2.tile([P, NB, H], BF16, tag="h2", name=f"h2_{it}")
        hc2 = []
        st2_t[it], h2_t[it] = st2, h2

        def unit(nb):
            ph = ps_h.tile([P, H + 1], F32, tag="h", name=f"p2_{it}_{nb}")
            for hh in range(2):
                nc.tensor.matmul(ph[:], ahT[hh][:, nb * P:(nb + 1) * P],
                                 w2_t[hh][:], start=(hh == 0), stop=(hh == 1))
            hc = pool_hc.tile([P, H], F32, tag="hc", name=f"hc2_{it}_{nb}")
            ln_stats(nb, ph, b2_t, sb2_t, st2, hc, f"2_{it}_{nb}")
            hc2.append(hc)

        def fin():
            finish_stats(st2)
            if fuse_pool:
                # last item: interleave mean-pool accumulation with the
                # applies so the PE doesn't sit out the whole LN2 tail;
                # applies alternate ACT / Pool to halve the serial chain
                pg = [ps_sm.tile([P, 1], F32, tag="h", name=f"pg_{it}_{kh}")
                      for kh in range(2)]
                for nb in range(NB):
                    apply_ln(nb, hc2[nb], st2, h2[:, nb, :],
                             gb_t.get("g2bc"), gb_t.get("be2bc"))
                    for kh in range(2):
                        nc.tensor.matmul(pg[kh][:],
                                         h2[:, nb, kh * P:(kh + 1) * P],
                                         ones_b[:], start=(nb == 0),
                                         stop=(nb == NB - 1))
                pg_t[it] = pg
            else:
                for nb in range(NB):
                    apply_ln(nb, hc2[nb], st2, h2[:, nb, :],
                             gb_t.get("g2bc"), gb_t.get("be2bc"))

        return unit, fin

    def pool_block(it):
        h2 = h2_t[it]
        gsb = pool_gsb.tile([P, 2], F32, tag="g", name=f"g_{it}")
        if pg_t[it] is None:
            pg = [ps_sm.tile([P, 1], F32, tag="h", name=f"pg_{it}_{kh}")
                  for kh in range(2)]
            for nb in range(NB):
                for kh in range(2):
                    nc.tensor.matmul(pg[kh][:],
                                     h2[:, nb, kh * P:(kh + 1) * P],
                                     ones_b[:], start=(nb == 0),
                                     stop=(nb == NB - 1))
        else:
            pg = pg_t[it]
        for kh in range(2):
            nc.scalar.mul(gsb[:, kh:kh + 1], pg[kh][:], 1.0 / N)

        for hd, (w_t, b_t, out_d) in enumerate(
                ((wa_t, ba_t, io["op"]), (wl_t, bl_t, io["ol"]))):
            po = ps_sm.tile([K, 1], F32, tag="h", name=f"po_{it}_{hd}")
            for kh in range(2):
                nc.tensor.matmul(po[:], w_t[kh][:], gsb[:, kh:kh + 1],
                                 start=(kh == 0), stop=(kh == 1))
            osb = pool_osb.tile([K, 1], F32, tag="o", name=f"o_{it}_{hd}")
            nc.scalar.activation(out=osb[:], in_=po[:],
                                 func=mybir.ActivationFunctionType.Identity,
                                 bias=b_t[:], scale=1.0)
            nc.sync.dma_start(out_d[it:it + 1, :], osb[:])

    # ---- software pipeline: dense phases interleave into the next agg
    # phase's c-loop; pool/head blocks trail their applies by one phase ----
    load(0, chunks=4)       # chunked so the first tiles land early
    emit_weight_dmas()
    if items == 1:
        l1_agg(0)
        unit, fin = make_l1_dense(0)
        for nb in range(NB):
            unit(nb)
        fin()
        l2_agg(0)
        unit, fin = make_l2_dense(0, fuse_pool=True)
        for nb in range(NB):
            unit(nb)
        fin()
        pool_block(0)
        es.close()
        return

    load(1)
    phases = [("l1", 0), ("l1", 1)]
    for it in range(items):
        phases.append(("l2", it))
        if it + 2 < items:
            phases.append(("l1", it + 2))

    ready = None            # (kind, it, unit, fin) pending dense phase
    pool_q = []             # items whose pool block is due next phase
    for kind, it in phases:
        pool_now, pool_q = pool_q, []
        cur, ready = ready, None
        unit = cur[2] if cur else None
        if kind == "l1":
            if it >= 2:
                load(it)
            l1_agg(it, unit)
        else:
            l2_agg(it, unit)
        if cur is not None:
            cur[3]()
            if cur[0] == "l2":
                pool_q.append(cur[1])
        for p in pool_now:
            pool_block(p)
        if kind == "l1":
            ready = ("l1", it) + make_l1_dense(it)
        else:
            ready = ("l2", it) + make_l2_dense(it, fuse_pool=(it == items - 1))

    # tail: the last item's dense phase has no agg left to hide in
    kind, itl, unit, fin = ready
    for nb in range(NB):
        unit(nb)
    for p in pool_q:
        pool_block(p)       # fills the finish_stats latency with PE work
    fin()
    pool_block(itl)

    es.close()


_CACHE = {}


def _get_nc(items, general):
    key = (items, general)
    if key not in _CACHE:
        nc = bacc.Bacc("TRN2", target_bir_lowering=False, debug=False,
                       num_devices=N_CORES)
        with tile.TileContext(nc) as tc:
            io = _declare_io(nc, items, general)
            _build_core(nc, tc, io, items, general)
        nc.compile()
        _CACHE[key] = nc
    return _CACHE[key]


def make_in_maps(A_hat, X, W1, b1, g1, beta1, W2, b2, g2, beta2,
                 Wa, ba, Wl, bl):
    """Host-side prep: shard over batch, transpose+cast A, fold gammas."""
    B = A_hat.shape[0]
    items = B // N_CORES
    general = bool(np.any(beta1 != 0) or np.any(beta2 != 0)
                   or np.any(g1 <= 0) or np.any(g2 <= 0))
    if general:
        w2f = np.asarray(W2, np.float32)
        waf = np.asarray(Wa, np.float32)
        wlf = np.asarray(Wl, np.float32)
    else:
        w2f = np.asarray(g1, np.float32)[:, None] * W2
        waf = (np.asarray(g2, np.float32)[:, None] * Wa).astype(np.float32)
        wlf = (np.asarray(g2, np.float32)[:, None] * Wl).astype(np.float32)
    w1f = np.asarray(W1, np.float32)
    w1e = np.concatenate([w1f, w1f.sum(1, keepdims=True)], 1)
    w2e = np.concatenate([w2f, w2f.sum(1, keepdims=True)], 1)
    shared = {
        "w1": w1e.astype(bf16),
        "w2": w2e.astype(bf16),
        "sb1": np.full((P, 1), np.float32(np.sum(np.asarray(b1, np.float32)))),
        "sb2": np.full((P, 1), np.float32(np.sum(np.asarray(b2, np.float32)))),
        "b1bc": np.ascontiguousarray(
            np.broadcast_to(np.asarray(b1, np.float32), (P, H))),
        "b2bc": np.ascontiguousarray(
            np.broadcast_to(np.asarray(b2, np.float32), (P, H))),
        "wa": waf, "wl": wlf,
        "ba": np.asarray(ba, np.float32).reshape(K, 1).copy(),
        "bl": np.asarray(bl, np.float32).reshape(K, 1).copy(),
        "ones": np.ones((P, 1), bf16),
    }
    if general:
        for nm, v in (("g1bc", g1), ("be1bc", beta1),
                      ("g2bc", g2), ("be2bc", beta2)):
            shared[nm] = np.ascontiguousarray(
                np.broadcast_to(np.asarray(v, np.float32), (P, H)))
    A_bf = np.asarray(A_hat, np.float32).astype(bf16)
    X_bf = np.asarray(X, np.float32).astype(bf16)
    in_maps = []
    for c in range(N_CORES):
        m = dict(shared)
        m["at4"] = np.ascontiguousarray(
            A_bf[c * items:(c + 1) * items].transpose(0, 2, 1))
        m["x4"] = np.ascontiguousarray(X_bf[c * items:(c + 1) * items])
        in_maps.append(m)
    return in_maps, items, general


def kernel(**inputs):
    in_maps, items, general = make_in_maps(**inputs)
    nc = _get_nc(items, general)
    res = run_bass_kernel_spmd(nc, in_maps, core_ids=list(range(N_CORES)))
    pred = np.concatenate([res.results[c]["op"] for c in range(N_CORES)], 0)
    logits = np.concatenate([res.results[c]["ol"] for c in range(N_CORES)], 0)
    return (np.asarray(pred, np.float32), np.asarray(logits, np.float32))


# revision 30
# speedup vs baseline: 2.4216x; 1.0197x over previous
"""TRN2 Bass kernel for nn_GCNBasic (2-layer GCN, B=32, N=2048, F=128, H=256).

Sharding: data-parallel over batch B across 8 NeuronCores (4 items/core);
small weights replicated.  A_hat is transposed and cast to bf16 on the HOST
(layout prep, halves HBM traffic); the device streams A^T tiles straight
into SBUF and runs pure matmul pipelines:

  (AX)^T[f,n]  = sum_mb  X[mb]-stationary   @ A^T[mb]   (c-outer, 4 psum
                                                         512-chunks live)
  H1pre[n,h]   = (AX)^T[:,nb]-stationary    @ W1
  H1           = relu(LN(H1pre + b1))        (bias-add Pool, sum DVE,
                                              sumsq+apply ACT, f32 stats)
  (AH)^T[hh,n] = sum_mb H1[mb,hh]-stationary @ A^T[mb]   (hh outer)
  H2pre[n,k]   = sum_hh (AH)^T[hh,nb]-stat.  @ diag(g1)W2
  H2           = relu(LN(H2pre + b2))
  g^T          = sum_nb H2[nb,kh]-stationary @ ones  (mean pool via PE)
  outputs      = diag(g2)Wa/Wl heads in fp32, biases added on ACT.

Items are software-pipelined (load it+2 / L1 of it+1 emitted between L2
and pool of it) so PE never waits on LayerNorm barriers or DMA refill.

gamma folds (diag(g1)@W2, diag(g2)@Wa/Wl) are exact because relu(g*z)=
g*relu(z) for g>0; beta==0 fast path (the problem's setup_inputs always
produces gamma=1, beta=0); a general gamma/beta path exists as a fallback.

Known TRN2 pitfalls worked around here: tensor_tensor_reduce crashes the
device; ACT/DVE writes into PSUM are unstable -> squares go to SBUF scratch.
"""

from contextlib import ExitStack

import numpy as np
import ml_dtypes

import concourse.bacc as bacc
import concourse.mybir as mybir
import concourse.tile as tile
from concourse.bass_utils import run_bass_kernel_spmd

F32 = mybir.dt.float32
BF16 = mybir.dt.bfloat16
bf16 = ml_dtypes.bfloat16

N = 2048
F = 128
H = 256
K = 64
P = 128
NB = N // P
NCH = N // 512
EPS = 1e-5
N_CORES = 8


def _declare_io(nc, items, general):
    io = {}
    io["at4"] = nc.dram_tensor("at4", [items, N, N], BF16, kind="ExternalInput")
    io["x4"] = nc.dram_tensor("x4", [items, N, F], BF16, kind="ExternalInput")
    # W1/W2 carry an extra trailing column holding W@1 so the dense matmul
    # also produces the LayerNorm row-sum (sans bias) as output column H.
    io["w1"] = nc.dram_tensor("w1", [F, H + 1], BF16, kind="ExternalInput")
    io["w2"] = nc.dram_tensor("w2", [H, H + 1], BF16, kind="ExternalInput")
    io["b1bc"] = nc.dram_tensor("b1bc", [P, H], F32, kind="ExternalInput")
    io["b2bc"] = nc.dram_tensor("b2bc", [P, H], F32, kind="ExternalInput")
    io["sb1"] = nc.dram_tensor("sb1", [P, 1], F32, kind="ExternalInput")
    io["sb2"] = nc.dram_tensor("sb2", [P, 1], F32, kind="ExternalInput")
    io["wa"] = nc.dram_tensor("wa", [H, K], F32, kind="ExternalInput")
    io["wl"] = nc.dram_tensor("wl", [H, K], F32, kind="ExternalInput")
    io["ba"] = nc.dram_tensor("ba", [K, 1], F32, kind="ExternalInput")
    io["bl"] = nc.dram_tensor("bl", [K, 1], F32, kind="ExternalInput")
    io["ones"] = nc.dram_tensor("ones", [P, 1], BF16, kind="ExternalInput")
    if general:
        io["g1bc"] = nc.dram_tensor("g1bc", [P, H], F32, kind="ExternalInput")
        io["be1bc"] = nc.dram_tensor("be1bc", [P, H], F32, kind="ExternalInput")
        io["g2bc"] = nc.dram_tensor("g2bc", [P, H], F32, kind="ExternalInput")
        io["be2bc"] = nc.dram_tensor("be2bc", [P, H], F32, kind="ExternalInput")
    io["op"] = nc.dram_tensor("op", [items, K], F32, kind="ExternalOutput")
    io["ol"] = nc.dram_tensor("ol", [items, K], F32, kind="ExternalOutput")
    return io


def _build_core(nc, tc, io, items, general):
    at4, x4 = io["at4"], io["x4"]
    es = ExitStack()

    consts = es.enter_context(tc.tile_pool(name="consts", bufs=1))
    wts = es.enter_context(tc.tile_pool(name="wts", bufs=1))
    pool_at = es.enter_context(tc.tile_pool(name="at", bufs=2 * NB))
    pool_xb = es.enter_context(tc.tile_pool(name="xb", bufs=2))
    pool_axT = es.enter_context(tc.tile_pool(name="axT", bufs=2))
    pool_h1 = es.enter_context(tc.tile_pool(name="h1", bufs=2))
    pool_ahT = es.enter_context(tc.tile_pool(name="ahT", bufs=2))
    pool_h2 = es.enter_context(tc.tile_pool(name="h2", bufs=1))
    pool_hc = es.enter_context(tc.tile_pool(name="hc", bufs=NB))
    pool_sq = es.enter_context(tc.tile_pool(name="sq", bufs=2))
    pool_st = es.enter_context(tc.tile_pool(name="st", bufs=4))
    pool_gsb = es.enter_context(tc.tile_pool(name="gsb", bufs=4))
    pool_osb = es.enter_context(tc.tile_pool(name="osb", bufs=4))

    ps_big = es.enter_context(tc.tile_pool(name="ps_big", bufs=6, space="PSUM"))
    ps_h = es.enter_context(tc.tile_pool(name="ps_h", bufs=2, space="PSUM"))
    ps_sm = ps_h  # pg/po share the ps_h banks (never live at the same time)

    eps_t = consts.tile([P, 1], F32)
    nc.vector.memset(eps_t[:], EPS)
    ones_b = consts.tile([P, 1], BF16)
    w1_t = wts.tile([P, H + 1], BF16)
    w2_t = [wts.tile([P, H + 1], BF16, tag=f"w2_{hh}", name=f"w2_{hh}")
            for hh in range(2)]
    b1_t = wts.tile([P, H], F32)
    b2_t = wts.tile([P, H], F32)
    sb1_t = wts.tile([P, 1], F32)
    sb2_t = wts.tile([P, 1], F32)
    wa_t = [wts.tile([P, K], F32, tag=f"wa_{hh}", name=f"wa_{hh}")
            for hh in range(2)]
    wl_t = [wts.tile([P, K], F32, tag=f"wl_{hh}", name=f"wl_{hh}")
            for hh in range(2)]
    ba_t = wts.tile([K, 1], F32)
    bl_t = wts.tile([K, 1], F32)
    gb_t = {}
    if general:
        for nm in ("g1bc", "be1bc", "g2bc", "be2bc"):
            gb_t[nm] = wts.tile([P, H], F32, tag=nm, name=nm)

    def emit_weight_dmas():
        nc.sync.dma_start(ones_b[:], io["ones"][:])
        nc.sync.dma_start(w1_t[:], io["w1"][:])
        for hh in range(2):
            nc.sync.dma_start(w2_t[hh][:], io["w2"][hh * P:(hh + 1) * P, :])
        nc.sync.dma_start(b1_t[:], io["b1bc"][:])
        nc.sync.dma_start(b2_t[:], io["b2bc"][:])
        nc.sync.dma_start(sb1_t[:], io["sb1"][:])
        nc.sync.dma_start(sb2_t[:], io["sb2"][:])
        for hh in range(2):
            nc.sync.dma_start(wa_t[hh][:], io["wa"][hh * P:(hh + 1) * P, :])
            nc.sync.dma_start(wl_t[hh][:], io["wl"][hh * P:(hh + 1) * P, :])
        nc.sync.dma_start(ba_t[:], io["ba"][:])
        nc.sync.dma_start(bl_t[:], io["bl"][:])
        for nm, t in gb_t.items():
            nc.sync.dma_start(t[:], io[nm][:])

    inv_h = 1.0 / H

    # per-item live tiles (indexed by item)
    at_t = [None] * items
    xb_t = [None] * items
    axT_t = [None] * items
    h1_t = [None] * items
    ahT_t = [None] * items
    h2_t = [None] * items
    hc1_t = [None] * items
    hc2_t = [None] * items
    st1_t = [None] * items
    st2_t = [None] * items

    def load(it, chunks=1):
        xb = pool_xb.tile([P, NB, F], BF16, tag="xb", name=f"xb_{it}")
        nc.sync.dma_start(xb[:], x4[it].rearrange("(c p) f -> p c f", p=P))
        xb_t[it] = xb
        ats = [pool_at.tile([P, N], BF16, tag="at", name=f"at_{it}_{c}")
               for c in range(NB)]
        cw = N // chunks
        for c in range(NB):
            for k in range(chunks):
                nc.sync.dma_start(
                    ats[c][:, k * cw:(k + 1) * cw],
                    at4[it, c * P:(c + 1) * P, k * cw:(k + 1) * cw])
        at_t[it] = ats

    def l1_agg(it, unit=None):
        # one pending dense unit (PE matmul + LN stats) dripped per c
        # iteration so the LN chain paces alongside pure agg matmuls
        at, xb = at_t[it], xb_t[it]
        pb = [ps_big.tile([P, 512], F32, tag="big", name=f"ax_{it}_{j}")
              for j in range(NCH)]
        for c in range(NB):
            for j in range(NCH):
                nc.tensor.matmul(pb[j][:], xb[:, c, :],
                                 at[c][:, j * 512:(j + 1) * 512],
                                 start=(c == 0), stop=(c == NB - 1))
            if unit is not None:
                unit(c)
        axT = pool_axT.tile([P, N], BF16, tag="axT", name=f"axT_{it}")
        for j in range(NCH):
            if j % 2 == 0:
                nc.scalar.copy(axT[:, j * 512:(j + 1) * 512], pb[j][:])
            else:
                nc.vector.tensor_copy(axT[:, j * 512:(j + 1) * 512], pb[j][:])
        axT_t[it] = axT

    def ln_stats(nb, ph, b_t, sb_t, st, hc, sfx):
        # bias add on DVE (PSUM->SBUF); row-sum comes from the matmul's
        # extra column H (+ bias total from sb); sumsq on Pool (one
        # scalar_tensor_tensor with accum_out -- keeps ACT free for applies)
        nc.vector.tensor_tensor(out=hc[:], in0=ph[:, 0:H], in1=b_t[:],
                                op=mybir.AluOpType.add)
        nc.vector.tensor_tensor(out=st[:, 0, nb:nb + 1], in0=ph[:, H:H + 1],
                                in1=sb_t[:], op=mybir.AluOpType.add)
        sq = pool_sq.tile([P, H], F32, tag="sq", name=f"sq_{sfx}")
        nc.scalar.activation(
            out=sq[:], in_=hc[:], func=mybir.ActivationFunctionType.Square,
            accum_out=st[:, 1, nb:nb + 1])

    def finish_stats(st):
        s = st
        nc.vector.tensor_scalar(out=s[:, 2, :], in0=s[:, 0, :],
                                scalar1=-inv_h, scalar2=None,
                                op0=mybir.AluOpType.mult)          # -mu
        nc.vector.tensor_tensor(out=s[:, 3, :], in0=s[:, 2, :], in1=s[:, 2, :],
                                op=mybir.AluOpType.mult)           # mu^2
        nc.vector.tensor_scalar(out=s[:, 4, :], in0=s[:, 1, :],
                                scalar1=inv_h, scalar2=None,
                                op0=mybir.AluOpType.mult)          # E[x^2]
        nc.vector.tensor_tensor(out=s[:, 4, :], in0=s[:, 4, :], in1=s[:, 3, :],
                                op=mybir.AluOpType.subtract)       # var
        nc.scalar.activation(out=s[:, 5, :], in_=s[:, 4, :],
                             func=mybir.ActivationFunctionType.Sqrt,
                             bias=eps_t[:], scale=1.0)             # sd
        nc.vector.reciprocal(out=s[:, 6, :], in_=s[:, 5, :])       # 1/sd
        nc.vector.tensor_tensor(out=s[:, 7, :], in0=s[:, 2, :], in1=s[:, 6, :],
                                op=mybir.AluOpType.mult)           # -mu/sd

    def apply_ln(nb, hc, st, h_out, g_bc, be_bc, use_act=True):
        if not general:
            if use_act:
                nc.scalar.activation(out=h_out, in_=hc[:],
                                     func=mybir.ActivationFunctionType.Relu,
                                     bias=st[:, 7, nb:nb + 1],
                                     scale=st[:, 6, nb:nb + 1])
            else:
                # Pool two-op apply keeps ACT free for the squares
                nc.gpsimd.tensor_scalar(out=hc[:], in0=hc[:],
                                        scalar1=st[:, 6, nb:nb + 1],
                                        scalar2=st[:, 7, nb:nb + 1],
                                        op0=mybir.AluOpType.mult,
                                        op1=mybir.AluOpType.add)
                nc.gpsimd.tensor_scalar_max(h_out, hc[:], 0.0)
        else:
            nc.scalar.activation(out=hc[:], in_=hc[:],
                                 func=mybir.ActivationFunctionType.Identity,
                                 bias=st[:, 7, nb:nb + 1],
                                 scale=st[:, 6, nb:nb + 1])
            nc.gpsimd.tensor_tensor(out=hc[:], in0=hc[:], in1=g_bc[:],
                                    op=mybir.AluOpType.mult)
            nc.vector.tensor_tensor(out=hc[:], in0=hc[:], in1=be_bc[:],
                                    op=mybir.AluOpType.add)
            nc.scalar.activation(out=h_out, in_=hc[:],
                                 func=mybir.ActivationFunctionType.Relu)

    def l2_agg(it, unit=None):
        at, h1 = at_t[it], h1_t[it]
        ahT = [pool_ahT.tile([P, N], BF16, tag="ahT", name=f"ahT_{it}_{hh}")
               for hh in range(2)]
        for hh in range(2):
            pb = [ps_big.tile([P, 512], F32, tag="big",
                              name=f"ah_{it}_{hh}_{j}") for j in range(NCH)]
            for c in range(NB):
                for j in range(NCH):
                    nc.tensor.matmul(pb[j][:], h1[:, c, hh * P:(hh + 1) * P],
                                     at[c][:, j * 512:(j + 1) * 512],
                                     start=(c == 0), stop=(c == NB - 1))
                if unit is not None and hh == 0:
                    unit(c)
            for j in range(NCH):
                if j % 2 == 0:
                    nc.scalar.copy(ahT[hh][:, j * 512:(j + 1) * 512], pb[j][:])
                else:
                    nc.vector.tensor_copy(
                        ahT[hh][:, j * 512:(j + 1) * 512], pb[j][:])
        ahT_t[it] = ahT

    pg_t = [None] * items

    def make_l1_dense(it):
        axT = axT_t[it]
        st1 = pool_st.tile([P, 8, NB], F32, tag="st", name=f"st1_{it}")
        h1 = pool_h1.tile([P, NB, H], BF16, tag="h1", name=f"h1_{it}")
        hc1 = []
        st1_t[it], h1_t[it] = st1, h1

        def unit(nb):
            ph = ps_h.tile([P, H + 1], F32, tag="h", name=f"p1_{it}_{nb}")
            nc.tensor.matmul(ph[:], axT[:, nb * P:(nb + 1) * P], w1_t[:],
                             start=True, stop=True)
            hc = pool_hc.tile([P, H], F32, tag="hc", name=f"hc1_{it}_{nb}")
            ln_stats(nb, ph, b1_t, sb1_t, st1, hc, f"1_{it}_{nb}")
            hc1.append(hc)

        def fin():
            finish_stats(st1)
            for nb in range(NB):
                apply_ln(nb, hc1[nb], st1, h1[:, nb, :],
                         gb_t.get("g1bc"), gb_t.get("be1bc"))

        return unit, fin

    def make_l2_dense(it, fuse_pool=False):
        ahT = ahT_t[it]
        st2 = pool_st.tile([P, 8, NB], F32, tag="st", name=f"st2_{it}")
        h2 = pool_h2.tile([P, NB, H], BF16, tag="h2", name=f"h2_{it}")
        hc2 = []
        st2_t[it], h2_t[it] = st2, h2

        def unit(nb):
            ph = ps_h.tile([P, H + 1], F32, tag="h", name=f"p2_{it}_{nb}")
            for hh in range(2):
                nc.tensor.matmul(ph[:], ahT[hh][:, nb * P:(nb + 1) * P],
                                 w2_t[hh][:], start=(hh == 0), stop=(hh == 1))
            hc = pool_hc.tile([P, H], F32, tag="hc", name=f"hc2_{it}_{nb}")
            ln_stats(nb, ph, b2_t, sb2_t, st2, hc, f"2_{it}_{nb}")
            hc2.append(hc)

        def fin():
            finish_stats(st2)
            if fuse_pool:
                # last item: interleave mean-pool accumulation with the
                # applies so the PE doesn't sit out the whole LN2 tail;
                # applies alternate ACT / Pool to halve the serial chain
                pg = [ps_sm.tile([P, 1], F32, tag="h", name=f"pg_{it}_{kh}")
                      for kh in range(2)]
                for nb in range(NB):
                    apply_ln(nb, hc2[nb], st2, h2[:, nb, :],
                             gb_t.get("g2bc"), gb_t.get("be2bc"))
                    for kh in range(2):
                        nc.tensor.matmul(pg[kh][:],
                                         h2[:, nb, kh * P:(kh + 1) * P],
                                         ones_b[:], start=(nb == 0),
                                         stop=(nb == NB - 1))
                pg_t[it] = pg
            else:
                for nb in range(NB):
                    apply_ln(nb, hc2[nb], st2, h2[:, nb, :],
                             gb_t.get("g2bc"), gb_t.get("be2bc"))

        return unit, fin

    def pool_block(it):
        h2 = h2_t[it]
        gsb = pool_gsb.tile([P, 2], F32, tag="g", name=f"g_{it}")
        if pg_t[it] is None:
            pg = [ps_sm.tile([P, 1], F32, tag="h", name=f"pg_{it}_{kh}")
                  for kh in range(2)]
            for nb in range(NB):
                for kh in range(2):
                    nc.tensor.matmul(pg[kh][:],
                                     h2[:, nb, kh * P:(kh + 1) * P],
                                     ones_b[:], start=(nb == 0),
                                     stop=(nb == NB - 1))
        else:
            pg = pg_t[it]
        for kh in range(2):
            nc.scalar.mul(gsb[:, kh:kh + 1], pg[kh][:], 1.0 / N)

        for hd, (w_t, b_t, out_d) in enumerate(
                ((wa_t, ba_t, io["op"]), (wl_t, bl_t, io["ol"]))):
            po = ps_sm.tile([K, 1], F32, tag="h", name=f"po_{it}_{hd}")
            for kh in range(2):
                nc.tensor.matmul(po[:], w_t[kh][:], gsb[:, kh:kh + 1],
                                 start=(kh == 0), stop=(kh == 1))
            osb = pool_osb.tile([K, 1], F32, tag="o", name=f"o_{it}_{hd}")
            nc.scalar.activation(out=osb[:], in_=po[:],
                                 func=mybir.ActivationFunctionType.Identity,
                                 bias=b_t[:], scale=1.0)
            nc.sync.dma_start(out_d[it:it + 1, :], osb[:])

    # ---- software pipeline: dense phases interleave into the next agg
    # phase's c-loop; pool/head blocks trail their applies by one phase ----
    load(0, chunks=4)       # chunked so the first tiles land early
    emit_weight_dmas()
    if items == 1:
        l1_agg(0)
        unit, fin = make_l1_dense(0)
        for nb in range(NB):
            unit(nb)
        fin()
        l2_agg(0)
        unit, fin = make_l2_dense(0, fuse_pool=True)
        for nb in range(NB):
            unit(nb)
        fin()
        pool_block(0)
        es.close()
        return

    load(1, chunks=2)
    phases = [("l1", 0), ("l1", 1)]
    for it in range(items):
        phases.append(("l2", it))
        if it + 2 < items:
            phases.append(("l1", it + 2))

    ready = None            # (kind, it, unit, fin) pending dense phase
    pool_q = []             # items whose pool block is due next phase
    for kind, it in phases:
        pool_now, pool_q = pool_q, []
        cur, ready = ready, None
        unit = cur[2] if cur else None
        if kind == "l1":
            if it >= 2:
                load(it, chunks=2)
            l1_agg(it, unit)
        else:
            l2_agg(it, unit)
        if cur is not None:
            cur[3]()
            if cur[0] == "l2":
                pool_q.append(cur[1])
        for p in pool_now:
            pool_block(p)
        if kind == "l1":
            ready = ("l1", it) + make_l1_dense(it)
        else:
            ready = ("l2", it) + make_l2_dense(it, fuse_pool=(it == items - 1))

    # tail: the last item's dense phase has no agg left to hide in
    kind, itl, unit, fin = ready
    for nb in range(NB):
        unit(nb)
    for p in pool_q:
        pool_block(p)       # fills the finish_stats latency with PE work
    fin()
    pool_block(itl)

    es.close()


_CACHE = {}


def _get_nc(items, general):
    key = (items, general)
    if key not in _CACHE:
        nc = bacc.Bacc("TRN2", target_bir_lowering=False, debug=False,
                       num_devices=N_CORES)
        with tile.TileContext(nc) as tc:
            io = _declare_io(nc, items, general)
            _build_core(nc, tc, io, items, general)
        nc.compile()
        _CACHE[key] = nc
    return _CACHE[key]


def make_in_maps(A_hat, X, W1, b1, g1, beta1, W2, b2, g2, beta2,
                 Wa, ba, Wl, bl):
    """Host-side prep: shard over batch, transpose+cast A, fold gammas."""
    B = A_hat.shape[0]
    items = B // N_CORES
    general = bool(np.any(beta1 != 0) or np.any(beta2 != 0)
                   or np.any(g1 <= 0) or np.any(g2 <= 0))
    if general:
        w2f = np.asarray(W2, np.float32)
        waf = np.asarray(Wa, np.float32)
        wlf = np.asarray(Wl, np.float32)
    else:
        w2f = np.asarray(g1, np.float32)[:, None] * W2
        waf = (np.asarray(g2, np.float32)[:, None] * Wa).astype(np.float32)
        wlf = (np.asarray(g2, np.float32)[:, None] * Wl).astype(np.float32)
    w1f = np.asarray(W1, np.float32)
    w1e = np.concatenate([w1f, w1f.sum(1, keepdims=True)], 1)
    w2e = np.concatenate([w2f, w2f.sum(1, keepdims=True)], 1)
    shared = {
        "w1": w1e.astype(bf16),
        "w2": w2e.astype(bf16),
        "sb1": np.full((P, 1), np.float32(np.sum(np.asarray(b1, np.float32)))),
        "sb2": np.full((P, 1), np.float32(np.sum(np.asarray(b2, np.float32)))),
        "b1bc": np.ascontiguousarray(
            np.broadcast_to(np.asarray(b1, np.float32), (P, H))),
        "b2bc": np.ascontiguousarray(
            np.broadcast_to(np.asarray(b2, np.float32), (P, H))),
        "wa": waf, "wl": wlf,
        "ba": np.asarray(ba, np.float32).reshape(K, 1).copy(),
        "bl": np.asarray(bl, np.float32).reshape(K, 1).copy(),
        "ones": np.ones((P, 1), bf16),
    }
    if general:
        for nm, v in (("g1bc", g1), ("be1bc", beta1),
                      ("g2bc", g2), ("be2bc", beta2)):
            shared[nm] = np.ascontiguousarray(
                np.broadcast_to(np.asarray(v, np.float32), (P, H)))
    A_bf = np.asarray(A_hat, np.float32).astype(bf16)
    X_bf = np.asarray(X, np.float32).astype(bf16)
    in_maps = []
    for c in range(N_CORES):
        m = dict(shared)
        m["at4"] = np.ascontiguousarray(
            A_bf[c * items:(c + 1) * items].transpose(0, 2, 1))
        m["x4"] = np.ascontiguousarray(X_bf[c * items:(c + 1) * items])
        in_maps.append(m)
    return in_maps, items, general


def kernel(**inputs):
    in_maps, items, general = make_in_maps(**inputs)
    nc = _get_nc(items, general)
    res = run_bass_kernel_spmd(nc, in_maps, core_ids=list(range(N_CORES)))
    pred = np.concatenate([res.results[c]["op"] for c in range(N_CORES)], 0)
    logits = np.concatenate([res.results[c]["ol"] for c in range(N_CORES)], 0)
    return (np.asarray(pred, np.float32), np.asarray(logits, np.float32))


# revision 31
# speedup vs baseline: 2.4470x; 1.0105x over previous
"""TRN2 Bass kernel for nn_GCNBasic (2-layer GCN, B=32, N=2048, F=128, H=256).

Sharding: data-parallel over batch B across 8 NeuronCores (4 items/core);
small weights replicated.  A_hat is transposed and cast to bf16 on the HOST
(layout prep, halves HBM traffic); the device streams A^T tiles straight
into SBUF and runs pure matmul pipelines:

  (AX)^T[f,n]  = sum_mb  X[mb]-stationary   @ A^T[mb]   (c-outer, 4 psum
                                                         512-chunks live)
  H1pre[n,h]   = (AX)^T[:,nb]-stationary    @ [W1 | W1@1]  (extra column =
                                              LN row-sum, free on the PE)
  H1           = relu(LN(H1pre + b1))        (bias-add DVE, sumsq+apply ACT,
                                              f32 stats)
  (AH)^T[hh,n] = sum_mb H1[mb,hh]-stationary @ A^T[mb]   (hh outer)
  H2pre[n,k]   = sum_hh (AH)^T[hh,nb]-stat.  @ [diag(g1)W2 | sumcol]
  H2           = relu(LN(H2pre + b2))
  g^T          = sum_nb H2[nb,kh]-stationary @ ones  (mean pool via PE)
  outputs      = diag(g2)Wa/Wl heads in fp32, biases added on ACT.

Items are software-pipelined at dense-matmul granularity: each dense
phase's 16 block-matmuls (+ their LN stats ops) are dripped one-per-c
into the NEXT aggregation phase's c-loop, so the PE never chases the
DVE/ACT LayerNorm chain through the 2 ps_h slots; pool/head blocks trail
their LN2 applies by one full phase; the last item fuses mean-pool
accumulation into its applies.  A^T tile DMAs are chunked across queues
(4-way for item 0, 2-way steady-state) to cut arrival latency.

gamma folds (diag(g1)@W2, diag(g2)@Wa/Wl) are exact because relu(g*z)=
g*relu(z) for g>0; beta==0 fast path (the problem's setup_inputs always
produces gamma=1, beta=0); a general gamma/beta path exists as a fallback.

Known TRN2 pitfalls worked around here: tensor_tensor_reduce crashes the
device; ACT/DVE writes into PSUM are unstable -> squares go to SBUF scratch.
"""

from contextlib import ExitStack

import numpy as np
import ml_dtypes

import concourse.bacc as bacc
import concourse.mybir as mybir
import concourse.tile as tile
from concourse.bass_utils import run_bass_kernel_spmd

F32 = mybir.dt.float32
BF16 = mybir.dt.bfloat16
bf16 = ml_dtypes.bfloat16

N = 2048
F = 128
H = 256
K = 64
P = 128
NB = N // P
NCH = N // 512
EPS = 1e-5
N_CORES = 8


def _declare_io(nc, items, general):
    io = {}
    io["at4"] = nc.dram_tensor("at4", [items, N, N], BF16, kind="ExternalInput")
    io["x4"] = nc.dram_tensor("x4", [items, N, F], BF16, kind="ExternalInput")
    # W1/W2 carry an extra trailing column holding W@1 so the dense matmul
    # also produces the LayerNorm row-sum (sans bias) as output column H.
    io["w1"] = nc.dram_tensor("w1", [F, H + 1], BF16, kind="ExternalInput")
    io["w2"] = nc.dram_tensor("w2", [H, H + 1], BF16, kind="ExternalInput")
    io["b1bc"] = nc.dram_tensor("b1bc", [P, H], F32, kind="ExternalInput")
    io["b2bc"] = nc.dram_tensor("b2bc", [P, H], F32, kind="ExternalInput")
    io["sb1"] = nc.dram_tensor("sb1", [P, 1], F32, kind="ExternalInput")
    io["sb2"] = nc.dram_tensor("sb2", [P, 1], F32, kind="ExternalInput")
    io["wa"] = nc.dram_tensor("wa", [H, K], F32, kind="ExternalInput")
    io["wl"] = nc.dram_tensor("wl", [H, K], F32, kind="ExternalInput")
    io["ba"] = nc.dram_tensor("ba", [K, 1], F32, kind="ExternalInput")
    io["bl"] = nc.dram_tensor("bl", [K, 1], F32, kind="ExternalInput")
    io["ones"] = nc.dram_tensor("ones", [P, 1], BF16, kind="ExternalInput")
    if general:
        io["g1bc"] = nc.dram_tensor("g1bc", [P, H], F32, kind="ExternalInput")
        io["be1bc"] = nc.dram_tensor("be1bc", [P, H], F32, kind="ExternalInput")
        io["g2bc"] = nc.dram_tensor("g2bc", [P, H], F32, kind="ExternalInput")
        io["be2bc"] = nc.dram_tensor("be2bc", [P, H], F32, kind="ExternalInput")
    io["op"] = nc.dram_tensor("op", [items, K], F32, kind="ExternalOutput")
    io["ol"] = nc.dram_tensor("ol", [items, K], F32, kind="ExternalOutput")
    return io


def _build_core(nc, tc, io, items, general):
    at4, x4 = io["at4"], io["x4"]
    es = ExitStack()

    consts = es.enter_context(tc.tile_pool(name="consts", bufs=1))
    wts = es.enter_context(tc.tile_pool(name="wts", bufs=1))
    pool_at = es.enter_context(tc.tile_pool(name="at", bufs=2 * NB))
    pool_xb = es.enter_context(tc.tile_pool(name="xb", bufs=2))
    pool_axT = es.enter_context(tc.tile_pool(name="axT", bufs=2))
    pool_h1 = es.enter_context(tc.tile_pool(name="h1", bufs=2))
    pool_ahT = es.enter_context(tc.tile_pool(name="ahT", bufs=2))
    pool_h2 = es.enter_context(tc.tile_pool(name="h2", bufs=1))
    pool_hc = es.enter_context(tc.tile_pool(name="hc", bufs=NB))
    pool_sq = es.enter_context(tc.tile_pool(name="sq", bufs=2))
    pool_st = es.enter_context(tc.tile_pool(name="st", bufs=4))
    pool_gsb = es.enter_context(tc.tile_pool(name="gsb", bufs=4))
    pool_osb = es.enter_context(tc.tile_pool(name="osb", bufs=4))

    ps_big = es.enter_context(tc.tile_pool(name="ps_big", bufs=6, space="PSUM"))
    ps_h = es.enter_context(tc.tile_pool(name="ps_h", bufs=2, space="PSUM"))
    ps_sm = ps_h  # pg/po share the ps_h banks (never live at the same time)

    eps_t = consts.tile([P, 1], F32)
    nc.vector.memset(eps_t[:], EPS)
    ones_b = consts.tile([P, 1], BF16)
    w1_t = wts.tile([P, H + 1], BF16)
    w2_t = [wts.tile([P, H + 1], BF16, tag=f"w2_{hh}", name=f"w2_{hh}")
            for hh in range(2)]
    b1_t = wts.tile([P, H], F32)
    b2_t = wts.tile([P, H], F32)
    sb1_t = wts.tile([P, 1], F32)
    sb2_t = wts.tile([P, 1], F32)
    wa_t = [wts.tile([P, K], F32, tag=f"wa_{hh}", name=f"wa_{hh}")
            for hh in range(2)]
    wl_t = [wts.tile([P, K], F32, tag=f"wl_{hh}", name=f"wl_{hh}")
            for hh in range(2)]
    ba_t = wts.tile([K, 1], F32)
    bl_t = wts.tile([K, 1], F32)
    gb_t = {}
    if general:
        for nm in ("g1bc", "be1bc", "g2bc", "be2bc"):
            gb_t[nm] = wts.tile([P, H], F32, tag=nm, name=nm)

    def emit_weight_dmas():
        nc.sync.dma_start(ones_b[:], io["ones"][:])
        nc.sync.dma_start(w1_t[:], io["w1"][:])
        for hh in range(2):
            nc.sync.dma_start(w2_t[hh][:], io["w2"][hh * P:(hh + 1) * P, :])
        nc.sync.dma_start(b1_t[:], io["b1bc"][:])
        nc.sync.dma_start(b2_t[:], io["b2bc"][:])
        nc.sync.dma_start(sb1_t[:], io["sb1"][:])
        nc.sync.dma_start(sb2_t[:], io["sb2"][:])
        for hh in range(2):
            nc.sync.dma_start(wa_t[hh][:], io["wa"][hh * P:(hh + 1) * P, :])
            nc.sync.dma_start(wl_t[hh][:], io["wl"][hh * P:(hh + 1) * P, :])
        nc.sync.dma_start(ba_t[:], io["ba"][:])
        nc.sync.dma_start(bl_t[:], io["bl"][:])
        for nm, t in gb_t.items():
            nc.sync.dma_start(t[:], io[nm][:])

    inv_h = 1.0 / H

    # per-item live tiles (indexed by item)
    at_t = [None] * items
    xb_t = [None] * items
    axT_t = [None] * items
    h1_t = [None] * items
    ahT_t = [None] * items
    h2_t = [None] * items
    hc1_t = [None] * items
    hc2_t = [None] * items
    st1_t = [None] * items
    st2_t = [None] * items

    def load(it, chunks=1):
        xb = pool_xb.tile([P, NB, F], BF16, tag="xb", name=f"xb_{it}")
        nc.sync.dma_start(xb[:], x4[it].rearrange("(c p) f -> p c f", p=P))
        xb_t[it] = xb
        ats = [pool_at.tile([P, N], BF16, tag="at", name=f"at_{it}_{c}")
               for c in range(NB)]
        cw = N // chunks
        for c in range(NB):
            for k in range(chunks):
                nc.sync.dma_start(
                    ats[c][:, k * cw:(k + 1) * cw],
                    at4[it, c * P:(c + 1) * P, k * cw:(k + 1) * cw])
        at_t[it] = ats

    def l1_agg(it, unit=None):
        # one pending dense unit (PE matmul + LN stats) dripped per c
        # iteration so the LN chain paces alongside pure agg matmuls
        at, xb = at_t[it], xb_t[it]
        pb = [ps_big.tile([P, 512], F32, tag="big", name=f"ax_{it}_{j}")
              for j in range(NCH)]
        for c in range(NB):
            for j in range(NCH):
                nc.tensor.matmul(pb[j][:], xb[:, c, :],
                                 at[c][:, j * 512:(j + 1) * 512],
                                 start=(c == 0), stop=(c == NB - 1))
            if unit is not None:
                unit(c)
        axT = pool_axT.tile([P, N], BF16, tag="axT", name=f"axT_{it}")
        for j in range(NCH):
            if j % 2 == 0:
                nc.scalar.copy(axT[:, j * 512:(j + 1) * 512], pb[j][:])
            else:
                nc.vector.tensor_copy(axT[:, j * 512:(j + 1) * 512], pb[j][:])
        axT_t[it] = axT

    def ln_stats(nb, ph, b_t, sb_t, st, hc, sfx):
        # bias add on DVE (PSUM->SBUF); row-sum comes from the matmul's
        # extra column H (+ bias total from sb); sumsq on Pool (one
        # scalar_tensor_tensor with accum_out -- keeps ACT free for applies)
        nc.vector.tensor_tensor(out=hc[:], in0=ph[:, 0:H], in1=b_t[:],
                                op=mybir.AluOpType.add)
        nc.vector.tensor_tensor(out=st[:, 0, nb:nb + 1], in0=ph[:, H:H + 1],
                                in1=sb_t[:], op=mybir.AluOpType.add)
        sq = pool_sq.tile([P, H], F32, tag="sq", name=f"sq_{sfx}")
        nc.scalar.activation(
            out=sq[:], in_=hc[:], func=mybir.ActivationFunctionType.Square,
            accum_out=st[:, 1, nb:nb + 1])

    def finish_stats(st):
        s = st
        nc.vector.tensor_scalar(out=s[:, 2, :], in0=s[:, 0, :],
                                scalar1=-inv_h, scalar2=None,
                                op0=mybir.AluOpType.mult)          # -mu
        nc.vector.tensor_tensor(out=s[:, 3, :], in0=s[:, 2, :], in1=s[:, 2, :],
                                op=mybir.AluOpType.mult)           # mu^2
        nc.vector.tensor_scalar(out=s[:, 4, :], in0=s[:, 1, :],
                                scalar1=inv_h, scalar2=None,
                                op0=mybir.AluOpType.mult)          # E[x^2]
        nc.vector.tensor_tensor(out=s[:, 4, :], in0=s[:, 4, :], in1=s[:, 3, :],
                                op=mybir.AluOpType.subtract)       # var
        nc.scalar.activation(out=s[:, 5, :], in_=s[:, 4, :],
                             func=mybir.ActivationFunctionType.Sqrt,
                             bias=eps_t[:], scale=1.0)             # sd
        nc.vector.reciprocal(out=s[:, 6, :], in_=s[:, 5, :])       # 1/sd
        nc.vector.tensor_tensor(out=s[:, 7, :], in0=s[:, 2, :], in1=s[:, 6, :],
                                op=mybir.AluOpType.mult)           # -mu/sd

    def apply_ln(nb, hc, st, h_out, g_bc, be_bc, use_act=True):
        if not general:
            if use_act:
                nc.scalar.activation(out=h_out, in_=hc[:],
                                     func=mybir.ActivationFunctionType.Relu,
                                     bias=st[:, 7, nb:nb + 1],
                                     scale=st[:, 6, nb:nb + 1])
            else:
                # Pool two-op apply keeps ACT free for the squares
                nc.gpsimd.tensor_scalar(out=hc[:], in0=hc[:],
                                        scalar1=st[:, 6, nb:nb + 1],
                                        scalar2=st[:, 7, nb:nb + 1],
                                        op0=mybir.AluOpType.mult,
                                        op1=mybir.AluOpType.add)
                nc.gpsimd.tensor_scalar_max(h_out, hc[:], 0.0)
        else:
            nc.scalar.activation(out=hc[:], in_=hc[:],
                                 func=mybir.ActivationFunctionType.Identity,
                                 bias=st[:, 7, nb:nb + 1],
                                 scale=st[:, 6, nb:nb + 1])
            nc.gpsimd.tensor_tensor(out=hc[:], in0=hc[:], in1=g_bc[:],
                                    op=mybir.AluOpType.mult)
            nc.vector.tensor_tensor(out=hc[:], in0=hc[:], in1=be_bc[:],
                                    op=mybir.AluOpType.add)
            nc.scalar.activation(out=h_out, in_=hc[:],
                                 func=mybir.ActivationFunctionType.Relu)

    def l2_agg(it, unit=None):
        at, h1 = at_t[it], h1_t[it]
        ahT = [pool_ahT.tile([P, N], BF16, tag="ahT", name=f"ahT_{it}_{hh}")
               for hh in range(2)]
        for hh in range(2):
            pb = [ps_big.tile([P, 512], F32, tag="big",
                              name=f"ah_{it}_{hh}_{j}") for j in range(NCH)]
            for c in range(NB):
                for j in range(NCH):
                    nc.tensor.matmul(pb[j][:], h1[:, c, hh * P:(hh + 1) * P],
                                     at[c][:, j * 512:(j + 1) * 512],
                                     start=(c == 0), stop=(c == NB - 1))
                if unit is not None and hh == 0:
                    unit(c)
            for j in range(NCH):
                if j % 2 == 0:
                    nc.scalar.copy(ahT[hh][:, j * 512:(j + 1) * 512], pb[j][:])
                else:
                    nc.vector.tensor_copy(
                        ahT[hh][:, j * 512:(j + 1) * 512], pb[j][:])
        ahT_t[it] = ahT

    pg_t = [None] * items

    def make_l1_dense(it):
        axT = axT_t[it]
        st1 = pool_st.tile([P, 8, NB], F32, tag="st", name=f"st1_{it}")
        h1 = pool_h1.tile([P, NB, H], BF16, tag="h1", name=f"h1_{it}")
        hc1 = []
        st1_t[it], h1_t[it] = st1, h1

        def unit(nb):
            ph = ps_h.tile([P, H + 1], F32, tag="h", name=f"p1_{it}_{nb}")
            nc.tensor.matmul(ph[:], axT[:, nb * P:(nb + 1) * P], w1_t[:],
                             start=True, stop=True)
            hc = pool_hc.tile([P, H], F32, tag="hc", name=f"hc1_{it}_{nb}")
            ln_stats(nb, ph, b1_t, sb1_t, st1, hc, f"1_{it}_{nb}")
            hc1.append(hc)

        def fin():
            finish_stats(st1)
            for nb in range(NB):
                apply_ln(nb, hc1[nb], st1, h1[:, nb, :],
                         gb_t.get("g1bc"), gb_t.get("be1bc"))

        return unit, fin

    def make_l2_dense(it, fuse_pool=False):
        ahT = ahT_t[it]
        st2 = pool_st.tile([P, 8, NB], F32, tag="st", name=f"st2_{it}")
        h2 = pool_h2.tile([P, NB, H], BF16, tag="h2", name=f"h2_{it}")
        hc2 = []
        st2_t[it], h2_t[it] = st2, h2

        def unit(nb):
            ph = ps_h.tile([P, H + 1], F32, tag="h", name=f"p2_{it}_{nb}")
            for hh in range(2):
                nc.tensor.matmul(ph[:], ahT[hh][:, nb * P:(nb + 1) * P],
                                 w2_t[hh][:], start=(hh == 0), stop=(hh == 1))
            hc = pool_hc.tile([P, H], F32, tag="hc", name=f"hc2_{it}_{nb}")
            ln_stats(nb, ph, b2_t, sb2_t, st2, hc, f"2_{it}_{nb}")
            hc2.append(hc)

        def fin():
            finish_stats(st2)
            if fuse_pool:
                # last item: interleave mean-pool accumulation with the
                # applies so the PE doesn't sit out the whole LN2 tail;
                # applies alternate ACT / Pool to halve the serial chain
                pg = [ps_sm.tile([P, 1], F32, tag="h", name=f"pg_{it}_{kh}")
                      for kh in range(2)]
                for nb in range(NB):
                    apply_ln(nb, hc2[nb], st2, h2[:, nb, :],
                             gb_t.get("g2bc"), gb_t.get("be2bc"))
                    for kh in range(2):
                        nc.tensor.matmul(pg[kh][:],
                                         h2[:, nb, kh * P:(kh + 1) * P],
                                         ones_b[:], start=(nb == 0),
                                         stop=(nb == NB - 1))
                pg_t[it] = pg
            else:
                for nb in range(NB):
                    apply_ln(nb, hc2[nb], st2, h2[:, nb, :],
                             gb_t.get("g2bc"), gb_t.get("be2bc"))

        return unit, fin

    def pool_block(it):
        h2 = h2_t[it]
        gsb = pool_gsb.tile([P, 2], F32, tag="g", name=f"g_{it}")
        if pg_t[it] is None:
            pg = [ps_sm.tile([P, 1], F32, tag="h", name=f"pg_{it}_{kh}")
                  for kh in range(2)]
            for nb in range(NB):
                for kh in range(2):
                    nc.tensor.matmul(pg[kh][:],
                                     h2[:, nb, kh * P:(kh + 1) * P],
                                     ones_b[:], start=(nb == 0),
                                     stop=(nb == NB - 1))
        else:
            pg = pg_t[it]
        for kh in range(2):
            nc.scalar.mul(gsb[:, kh:kh + 1], pg[kh][:], 1.0 / N)

        for hd, (w_t, b_t, out_d) in enumerate(
                ((wa_t, ba_t, io["op"]), (wl_t, bl_t, io["ol"]))):
            po = ps_sm.tile([K, 1], F32, tag="h", name=f"po_{it}_{hd}")
            for kh in range(2):
                nc.tensor.matmul(po[:], w_t[kh][:], gsb[:, kh:kh + 1],
                                 start=(kh == 0), stop=(kh == 1))
            osb = pool_osb.tile([K, 1], F32, tag="o", name=f"o_{it}_{hd}")
            nc.scalar.activation(out=osb[:], in_=po[:],
                                 func=mybir.ActivationFunctionType.Identity,
                                 bias=b_t[:], scale=1.0)
            nc.sync.dma_start(out_d[it:it + 1, :], osb[:])

    # ---- software pipeline: dense phases interleave into the next agg
    # phase's c-loop; pool/head blocks trail their applies by one phase ----
    load(0, chunks=4)       # chunked so the first tiles land early
    emit_weight_dmas()
    if items == 1:
        l1_agg(0)
        unit, fin = make_l1_dense(0)
        for nb in range(NB):
            unit(nb)
        fin()
        l2_agg(0)
        unit, fin = make_l2_dense(0, fuse_pool=True)
        for nb in range(NB):
            unit(nb)
        fin()
        pool_block(0)
        es.close()
        return

    load(1, chunks=2)
    phases = [("l1", 0), ("l1", 1)]
    for it in range(items):
        phases.append(("l2", it))
        if it + 2 < items:
            phases.append(("l1", it + 2))

    ready = None            # (kind, it, unit, fin) pending dense phase
    pool_q = []             # items whose pool block is due next phase
    for kind, it in phases:
        pool_now, pool_q = pool_q, []
        cur, ready = ready, None
        unit = cur[2] if cur else None
        if kind == "l1":
            if it >= 2:
                load(it, chunks=2)
            l1_agg(it, unit)
        else:
            l2_agg(it, unit)
        if cur is not None:
            cur[3]()
            if cur[0] == "l2":
                pool_q.append(cur[1])
        for p in pool_now:
            pool_block(p)
        if kind == "l1":
            ready = ("l1", it) + make_l1_dense(it)
        else:
            ready = ("l2", it) + make_l2_dense(it, fuse_pool=(it == items - 1))

    # tail: the last item's dense phase has no agg left to hide in
    kind, itl, unit, fin = ready
    for nb in range(NB):
        unit(nb)
    for p in pool_q:
        pool_block(p)       # fills the finish_stats latency with PE work
    fin()
    pool_block(itl)

    es.close()


_CACHE = {}


def _get_nc(items, general):
    key = (items, general)
    if key not in _CACHE:
        nc = bacc.Bacc("TRN2", target_bir_lowering=False, debug=False,
                       num_devices=N_CORES)
        with tile.TileContext(nc) as tc:
            io = _declare_io(nc, items, general)
            _build_core(nc, tc, io, items, general)
        nc.compile()
        _CACHE[key] = nc
    return _CACHE[key]


def make_in_maps(A_hat, X, W1, b1, g1, beta1, W2, b2, g2, beta2,
                 Wa, ba, Wl, bl):
    """Host-side prep: shard over batch, transpose+cast A, fold gammas."""
    B = A_hat.shape[0]
    items = B // N_CORES
    general = bool(np.any(beta1 != 0) or np.any(beta2 != 0)
                   or np.any(g1 <= 0) or np.any(g2 <= 0))
    if general:
        w2f = np.asarray(W2, np.float32)
        waf = np.asarray(Wa, np.float32)
        wlf = np.asarray(Wl, np.float32)
    else:
        w2f = np.asarray(g1, np.float32)[:, None] * W2
        waf = (np.asarray(g2, np.float32)[:, None] * Wa).astype(np.float32)
        wlf = (np.asarray(g2, np.float32)[:, None] * Wl).astype(np.float32)
    w1f = np.asarray(W1, np.float32)
    w1e = np.concatenate([w1f, w1f.sum(1, keepdims=True)], 1)
    w2e = np.concatenate([w2f, w2f.sum(1, keepdims=True)], 1)
    shared = {
        "w1": w1e.astype(bf16),
        "w2": w2e.astype(bf16),
        "sb1": np.full((P, 1), np.float32(np.sum(np.asarray(b1, np.float32)))),
        "sb2": np.full((P, 1), np.float32(np.sum(np.asarray(b2, np.float32)))),
        "b1bc": np.ascontiguousarray(
            np.broadcast_to(np.asarray(b1, np.float32), (P, H))),
        "b2bc": np.ascontiguousarray(
            np.broadcast_to(np.asarray(b2, np.float32), (P, H))),
        "wa": waf, "wl": wlf,
        "ba": np.asarray(ba, np.float32).reshape(K, 1).copy(),
        "bl": np.asarray(bl, np.float32).reshape(K, 1).copy(),
        "ones": np.ones((P, 1), bf16),
    }
    if general:
        for nm, v in (("g1bc", g1), ("be1bc", beta1),
                      ("g2bc", g2), ("be2bc", beta2)):
            shared[nm] = np.ascontiguousarray(
                np.broadcast_to(np.asarray(v, np.float32), (P, H)))
    A_bf = np.asarray(A_hat, np.float32).astype(bf16)
    X_bf = np.asarray(X, np.float32).astype(bf16)
    in_maps = []
    for c in range(N_CORES):
        m = dict(shared)
        m["at4"] = np.ascontiguousarray(
            A_bf[c * items:(c + 1) * items].transpose(0, 2, 1))
        m["x4"] = np.ascontiguousarray(X_bf[c * items:(c + 1) * items])
        in_maps.append(m)
    return in_maps, items, general


def kernel(**inputs):
    in_maps, items, general = make_in_maps(**inputs)
    nc = _get_nc(items, general)
    res = run_bass_kernel_spmd(nc, in_maps, core_ids=list(range(N_CORES)))
    pred = np.concatenate([res.results[c]["op"] for c in range(N_CORES)], 0)
    logits = np.concatenate([res.results[c]["ol"] for c in range(N_CORES)], 0)
    return (np.asarray(pred, np.float32), np.asarray(logits, np.float32))
